# revision 6
# baseline (speedup 1.0000x reference)
"""Trainium2 Bass kernel for the isotropic-gaussian differentiable renderer.

Math: for pixel p=(x,y) and gaussian g:
    w[g,p] = op_g * exp(-0.5*((x-ax_g)^2+(y-ay_g)^2)/var_g)
    img[p,c] = (sum_g w[g,p]*col_gc) / (sum_g w[g,p] + n_chunks*EPS)

The isotropic RBF is separable: w = exp(sx) * B with
sx = s*(x-ax)^2, B = op*exp(s*(y-ay)^2), s = -0.5/var.  Per 128-gaussian
chunk:

  PE (fp16): arg[g, 0:128]=sx(g,x), arg[g,128:256]=sy(g,y)+ln(op) via a
             K=11 matmul against fixed basis rows.  fp16 stays exact
             because the quadratic basis u^2 is split into b1 (fp16-exact)
             + b2 (residual in {-1,0,1}) rows and every coefficient is a
             hi/lo fp16 pair (lo of the shared s row pre-scaled by 2^12 to
             stay in normal range); the catastrophically-cancelling
             quadratic matches fp64 to ~2e-5.
  ACT      : exp(arg) -> fp16 into fused per-chunk blocks
             [expx(128) | B(128) | colors(384)]; the y half lands as the
             den block B = op*expy directly (ln(op) is in the argument)
  DVE      : ONE broadcast tensor_tensor per chunk fills all 3 color
             blocks: B (stride-0 broadcast x3) * colrep (host-replicated
             [r*128|g*128|b*128] per gaussian).  One dispatch instead of
             three keeps the Vector engine far off the critical path.
  PE (fp16): acc += block[0:128]^T @ block[128:640] (fp32 PSUM
             accumulate); chunks 0-7 into accA, 8-15 into accB so accA's
             result DMA overlaps the tail of compute (host sums partials)

The PE is warmed with dense dummy matmuls from ~t=0.3us that connect
seamlessly into the real matmul stream: the HAM clock gate needs one
fully-busy 3413ns window before it opens to 2.4GHz, and any idle gap
re-arms the throttle.

Sharding: gaussians split 2048/core across 8 cores; every core accumulates
the full 128x128 image; host sums the 16 partials (2 per core), divides
num/den and reshapes to the reference's [4,3,64,64] tile layout.
"""
import numpy as np

import concourse.bacc as bacc
import concourse.tile as tile
from concourse import mybir
from concourse.bass_utils import run_bass_kernel_spmd

# Problem constants (hardcoded per harness contract)
N_GAUSS = 16384
H = 128
W = 128
FX = 128.0
FY = 128.0
CX = 64.0
CY = 64.0
EPS = 1e-8
N_CORES = 8
G_PER_CORE = N_GAUSS // N_CORES      # 2048
CHUNK = 128                          # gaussians per matmul chunk
N_CHUNKS = G_PER_CORE // CHUNK       # 16
ARG_W = 256                          # per-chunk arg width: 128 x | 128 y
OUT_W = 512                          # (c,y) free width of one accumulator

F32 = mybir.dt.float32
MM_DT = mybir.dt.float16             # block dtype: fp16 rounding of B is
# shared by num and den so it cancels in the ratio; colors carry an
# independent 2^-11 rounding which averages out over gaussians.
KARG = 11                            # arg-matmul contraction rows
N_WARM = 12                          # dense PE warmup matmuls (HAM gate)
ONE_TT = True                        # one bcast tensor_tensor vs 3 muls
BLK = 640                            # per-chunk fused block width
COEF_W = G_PER_CORE + ARG_W          # packed [coef | basis] columns


def build_program():
    """One SPMD Bass program; every core runs it on its gaussian slice."""
    nc = bacc.Bacc("TRN2", target_bir_lowering=False, debug=False,
                   num_devices=N_CORES)
    # [11, 2048+256] fp16: stationary coefficient rows (one chunk per 128
    # cols) packed with the 256 fixed basis cols so ONE DMA delivers both
    coefrhs = nc.dram_tensor("coefrhs", [KARG, COEF_W], MM_DT,
                             kind="ExternalInput")
    # [128, 16*384] fp16: colrep[p, c*384 + b*128 + y] = col[c*128+p, b]
    # (host-replicated so the color build is one broadcast multiply)
    colrep = nc.dram_tensor("colrep", [128, N_CHUNKS * 384], MM_DT,
                            kind="ExternalInput")
    # [128, 64] fp32 per-gaussian color scalars (fallback 3-mul path)
    opc = nc.dram_tensor("opc", [128, N_CHUNKS * 4], F32,
                         kind="ExternalInput")
    # two partial accumulators: [x, (den|r|g|b)*128+y] each
    out = nc.dram_tensor("out", [128, 2 * OUT_W], F32, kind="ExternalOutput")

    with tile.TileContext(nc) as tc:
        with tc.tile_pool(name="ins", bufs=1) as ins_pool, \
             tc.tile_pool(name="expp", bufs=1) as exp_pool, \
             tc.tile_pool(name="args", bufs=3, space="PSUM") as arg_pool, \
             tc.tile_pool(name="acc", bufs=1, space="PSUM") as acc_pool, \
             tc.tile_pool(name="outp", bufs=1) as out_pool:

            cr_t = ins_pool.tile([KARG, COEF_W], MM_DT)
            colrep_t = ins_pool.tile([128, N_CHUNKS, 3, 128], MM_DT)
            opc_t = ins_pool.tile([128, N_CHUNKS * 4], F32)

            # Warmup source memset first so the PE can start immediately;
            # input DMAs issue in parallel on rings that keep the Scalar
            # engine free early for its activation-table load.
            wsrc = ins_pool.tile([128, ARG_W], mybir.dt.bfloat16)
            nc.gpsimd.memset(wsrc, 0.0)
            nc.sync.dma_start(out=cr_t, in_=coefrhs[:, :])
            Q = N_CHUNKS // 4
            # colrep quarters interleaved across the scalar/gpsimd rings in
            # need-order: chunks 0-3 first (scalar), 4-7 (gpsimd), ...
            for qi in range(4):
                eng = nc.scalar if qi % 2 == 0 else nc.gpsimd
                if ONE_TT:
                    eng.dma_start(
                        out=colrep_t[:, qi * Q:(qi + 1) * Q, :, :],
                        in_=colrep[:, qi * Q * 384:(qi + 1) * Q * 384],
                    )
            if not ONE_TT:
                nc.gpsimd.dma_start(out=opc_t, in_=opc[:, :])

            # fused per-chunk block [expx(128) | B(128) | colors(384)]
            t3 = exp_pool.tile([128, N_CHUNKS, BLK], MM_DT)
            accA = acc_pool.tile([128, OUT_W], F32)
            accB = acc_pool.tile([128, OUT_W], F32)

            # Dense PE warmup into accA (overwritten by main0's start=True):
            # HAM opens the clock gate to 8/8 (2.4GHz) only after a
            # fully-busy free-running 3413ns window; these run while the
            # input DMAs are in flight and hand off to the real stream.
            for _ in range(N_WARM):
                nc.tensor.matmul(accA[:, :ARG_W], wsrc[:, :CHUNK],
                                 wsrc[:, :], start=True, stop=True)

            rhs0 = G_PER_CORE                      # basis cols offset
            group_plan = [(0, 1), (1, 1), (2, 2), (4, 4), (8, 4), (12, 4)]

            def issue_args(g0c, width):
                args = arg_pool.tile([128, width * ARG_W], F32, tag="args")
                for k in range(width):
                    chunk = g0c + k
                    nc.tensor.matmul(
                        args[:, k * ARG_W:(k + 1) * ARG_W],
                        cr_t[:, chunk * CHUNK:(chunk + 1) * CHUNK],
                        cr_t[:, rhs0:rhs0 + ARG_W],
                        start=True, stop=True,
                    )
                nc.scalar.activation(
                    out=t3[:, g0c:g0c + width, 0:ARG_W],
                    in_=args[:, :width * ARG_W],
                    func=mybir.ActivationFunctionType.Exp,
                )
                for k in range(width):
                    chunk = g0c + k
                    # y half of the exp is B = op*expy (ln(op) in the
                    # arg); color blocks multiply the SAME rounded B so
                    # num/den rounding cancels.  Column order: [den|r|g|b]
                    if ONE_TT:
                        b3 = t3[:, chunk, 128:256].unsqueeze(1) \
                            .broadcast_to([128, 3, 128])
                        o3 = t3[:, chunk, 256:640].rearrange(
                            "p (c y) -> p c y", c=3)
                        nc.vector.tensor_tensor(
                            out=o3, in0=b3, in1=colrep_t[:, chunk, :, :],
                            op=mybir.AluOpType.mult,
                        )
                    else:
                        for c in range(3):
                            nc.vector.tensor_scalar_mul(
                                out=t3[:, chunk, 256 + c * 128:
                                       256 + (c + 1) * 128],
                                in0=t3[:, chunk, 128:256],
                                scalar1=opc_t[:, chunk * 4 + c:
                                              chunk * 4 + c + 1],
                            )

            def issue_main(chunk):
                acc = accA if chunk < 8 else accB
                nc.tensor.matmul(
                    acc[:, :],
                    t3[:, chunk, 0:128],
                    t3[:, chunk, 128:BLK],
                    start=(chunk % 8 == 0), stop=(chunk % 8 == 7),
                )

            # PE program order: args run 1-2 groups ahead of mains so the
            # engine queue always has work and never re-arms the throttle
            issue_args(0, 1)
            issue_args(1, 1)
            issue_args(2, 2)
            issue_main(0)
            issue_args(4, 4)
            for c in (1, 2, 3):
                issue_main(c)
            issue_args(8, 4)
            for c in (4, 5, 6, 7):
                issue_main(c)
            issue_args(12, 4)
            for c in range(8, 16):
                issue_main(c)

            out_t = out_pool.tile([128, 2 * OUT_W], F32)
            # accA final after main7: its copy+DMA overlap tail compute
            nc.scalar.copy(out=out_t[:, :OUT_W], in_=accA[:, :])
            nc.sync.dma_start(out=out[:, :OUT_W], in_=out_t[:, :OUT_W])
            # accB: split halves across engines/rings to shorten the tail
            nc.scalar.copy(out=out_t[:, OUT_W:OUT_W + 256],
                           in_=accB[:, :256])
            nc.vector.tensor_copy(out_t[:, OUT_W + 256:], accB[:, 256:])
            nc.sync.dma_start(out=out[:, OUT_W:OUT_W + 256],
                              in_=out_t[:, OUT_W:OUT_W + 256])
            nc.gpsimd.dma_start(out=out[:, OUT_W + 256:],
                                in_=out_t[:, OUT_W + 256:])

    nc.compile()
    return nc


_PROGRAM = None


def _get_program():
    global _PROGRAM
    if _PROGRAM is None:
        _PROGRAM = build_program()
    return _PROGRAM


def _quat2mat(q):
    q = q / np.linalg.norm(q)
    w, x, y, z = q
    return np.array([
        [1 - 2 * (y * y + z * z), 2 * (x * y - z * w), 2 * (x * z + y * w)],
        [2 * (x * y + z * w), 1 - 2 * (x * x + z * z), 2 * (y * z - x * w)],
        [2 * (x * z - y * w), 2 * (y * z + x * w), 1 - 2 * (x * x + y * y)],
    ])


def _hilo16(x):
    """Split x (f64) into fp16-representable hi+lo with hi+lo ~= x."""
    hi = np.asarray(x, dtype=np.float16)
    lo = (np.asarray(x, dtype=np.float64) - hi.astype(np.float64)) \
        .astype(np.float16)
    return hi, lo


def kernel(positions, colors, opacities, scales, qvec, tvec, tile_hw,
           chunk_gauss, _trace=False):
    positions = np.asarray(positions, dtype=np.float32)
    colors = np.asarray(colors, dtype=np.float32)
    opacities = np.asarray(opacities, dtype=np.float32)
    scales = np.asarray(scales, dtype=np.float32)
    qvec = np.asarray(qvec, dtype=np.float32)
    tvec = np.asarray(tvec, dtype=np.float32)
    tile_hw = int(tile_hw)
    chunk_gauss = int(chunk_gauss)
    n = positions.shape[0]
    assert n == N_GAUSS, f"expected {N_GAUSS} gaussians, got {n}"

    # ---- O(N) per-gaussian prep in float64 (rounds to the same f32 values
    # the reference computes, to well within the exp's own error budget) ----
    R = _quat2mat(qvec.astype(np.float64))
    cam = positions.astype(np.float64) @ R.T + tvec.astype(np.float64)
    ax = cam[:, 0] / cam[:, 2] * FX + CX          # [N] screen x center
    ay = cam[:, 1] / cam[:, 2] * FY + CY          # [N] screen y center
    var = scales[:, 0].astype(np.float64) ** 2
    s = -0.5 / var                                # [N] negative inv 2*var

    # centered coords keep the quadratic-expansion terms small (|u|<=64)
    dx = ax - CX
    dy = ay - CY
    op64 = opacities[:, 0].astype(np.float64)

    # K=11 stationary rows per gaussian (fp16), for
    #   arg_x = s*u^2 + (-2 s dx)*u + s*dx^2            (u = x - 64)
    #   arg_y = s*v^2 + (-2 s dy)*v + s*dy^2 + ln(op)   (v = y - 64)
    # The u^2 basis is split into b1 = fp16(u^2) (exact products against
    # hi/lo halves of s) and the residual b2 = u^2-b1 in {-1,0,1} handled
    # by a single-precision s row; s_lo is pre-scaled by 2^12 (and its
    # basis row by 2^-12) to stay inside fp16 normal range.
    s_hi = s.astype(np.float16)
    s_lo12 = ((s - s_hi.astype(np.float64)) * 4096.0).astype(np.float16)
    mx_hi, mx_lo = _hilo16(-2.0 * s * dx)
    my_hi, my_lo = _hilo16(-2.0 * s * dy)
    cx_hi, cx_lo = _hilo16(s * dx * dx)
    cy_hi, cy_lo = _hilo16(s * dy * dy + np.log(op64))
    coef_full = np.stack([s_hi, s_lo12, s_hi,
                          mx_hi, mx_lo, cx_hi, cx_lo,
                          my_hi, my_lo, cy_hi, cy_lo])   # [11, N] fp16

    u = np.arange(W, dtype=np.float64) - CX
    v = np.arange(H, dtype=np.float64) - CY
    b1u = (u * u).astype(np.float16).astype(np.float64)
    b2u = u * u - b1u
    b1v = (v * v).astype(np.float16).astype(np.float64)
    b2v = v * v - b1v
    zer = np.zeros(128)
    one = np.ones(128)
    rhs_rows = [
        np.concatenate([b1u, b1v]),                   # s_hi
        np.concatenate([b1u, b1v]) / 4096.0,          # s_lo12
        np.concatenate([b2u, b2v]),                   # s_hi (residual row)
        np.concatenate([u, zer]),                     # mx_hi
        np.concatenate([u, zer]),                     # mx_lo
        np.concatenate([one, zer]),                   # cx_hi
        np.concatenate([one, zer]),                   # cx_lo
        np.concatenate([zer, v]),                     # my_hi
        np.concatenate([zer, v]),                     # my_lo
        np.concatenate([zer, one]),                   # cy_hi
        np.concatenate([zer, one]),                   # cy_lo
    ]
    rhsxy = np.stack(rhs_rows).astype(np.float16)     # [11, 256]

    col16 = colors.astype(np.float16)                 # [N, 3]
    opc_full = np.concatenate(
        [colors, np.ones((n, 1), np.float32)], axis=1).astype(np.float32)

    # ---- shard gaussians across the 8 cores ----
    in_maps = []
    for core in range(N_CORES):
        g0 = core * G_PER_CORE
        g1 = g0 + G_PER_CORE
        coefrhs = np.concatenate(
            [coef_full[:, g0:g1], rhsxy], axis=1)     # [11, 2304]
        # colrep[p, chunk, c, y] = col16[g0 + chunk*128 + p, c]
        cc = col16[g0:g1].reshape(N_CHUNKS, CHUNK, 3)
        colrep_c = np.ascontiguousarray(
            np.broadcast_to(cc.transpose(1, 0, 2)[:, :, :, None],
                            (CHUNK, N_CHUNKS, 3, 128))
            .reshape(CHUNK, N_CHUNKS * 384))
        opc_c = opc_full[g0:g1].reshape(N_CHUNKS, CHUNK, 4)
        opc_c = np.ascontiguousarray(
            opc_c.transpose(1, 0, 2).reshape(CHUNK, N_CHUNKS * 4))
        in_maps.append({
            "coefrhs": np.ascontiguousarray(coefrhs),
            "colrep": colrep_c,
            "opc": opc_c,
        })

    nc = _get_program()
    res = run_bass_kernel_spmd(nc, in_maps, list(range(N_CORES)),
                               trace=_trace)

    # ---- host reduction: sum per-core partials, divide, reshape ----
    acc = np.zeros((128, 4, 128), dtype=np.float64)   # [x, (den|r|g|b), y]
    for core in range(N_CORES):
        o = res.results[core]["out"]
        acc += o[:, :OUT_W].reshape(128, 4, 128)
        acc += o[:, OUT_W:].reshape(128, 4, 128)

    num = acc[:, 1:4, :]                          # [x, c, y]
    n_chunks_ref = n // chunk_gauss
    den = acc[:, 0, :] + n_chunks_ref * EPS       # [x, y]
    img = num / den[:, None, :]                   # [x, c, y]
    img = img.transpose(2, 0, 1).reshape(H * W, 3)  # [p=(y,x), c]

    step = tile_hw * tile_hw
    t = (H * W) // step
    out = img.reshape(t, step, 3).transpose(0, 2, 1).reshape(
        t, 3, tile_hw, tile_hw)
    result = out.astype(np.float32)
    if _trace:
        return result, res
    return result


# revision 8
# speedup vs baseline: 1.0379x; 1.0379x over previous
"""Trainium2 Bass kernel for the isotropic-gaussian differentiable renderer.

Math: for pixel p=(x,y) and gaussian g:
    w[g,p] = op_g * exp(-0.5*((x-ax_g)^2+(y-ay_g)^2)/var_g)
    img[p,c] = (sum_g w[g,p]*col_gc) / (sum_g w[g,p] + n_chunks*EPS)

The isotropic RBF is separable: w = exp(sx) * B with
sx = s*(x-ax)^2, B = op*exp(s*(y-ay)^2), s = -0.5/var.  Per 128-gaussian
chunk:

  PE (fp16): arg[g, 0:128]=sx(g,x), arg[g,128:256]=sy(g,y)+ln(op) via a
             K=11 matmul against fixed basis rows.  fp16 stays exact
             because the quadratic basis u^2 is split into b1 (fp16-exact)
             + b2 (residual in {-1,0,1}) rows and every coefficient is a
             hi/lo fp16 pair (lo of the shared s row pre-scaled by 2^12 to
             stay in normal range); the catastrophically-cancelling
             quadratic matches fp64 to ~2e-5.
  ACT      : exp(arg) -> fp16 into fused per-chunk blocks
             [expx(128) | B(128) | colors(384)]; the y half lands as the
             den block B = op*expy directly (ln(op) is in the argument)
  DVE      : ONE broadcast tensor_tensor per chunk fills all 3 color
             blocks: B (stride-0 broadcast x3) * colrep (host-replicated
             [r*128|g*128|b*128] per gaussian).  One dispatch instead of
             three keeps the Vector engine far off the critical path.
  PE (fp16): acc += block[0:128]^T @ block[128:640] (fp32 PSUM
             accumulate); chunks 0-7 into accA, 8-15 into accB so accA's
             result DMA overlaps the tail of compute (host sums partials)

The PE is warmed with dense dummy matmuls from ~t=0.3us that connect
seamlessly into the real matmul stream: the HAM clock gate needs one
fully-busy 3413ns window before it opens to 2.4GHz, and any idle gap
re-arms the throttle.

Sharding: gaussians split 2048/core across 8 cores; every core accumulates
the full 128x128 image; host sums the 16 partials (2 per core), divides
num/den and reshapes to the reference's [4,3,64,64] tile layout.
"""
import numpy as np

import concourse.bacc as bacc
import concourse.tile as tile
from concourse import mybir
from concourse.bass_utils import run_bass_kernel_spmd

# Problem constants (hardcoded per harness contract)
N_GAUSS = 16384
H = 128
W = 128
FX = 128.0
FY = 128.0
CX = 64.0
CY = 64.0
EPS = 1e-8
N_CORES = 8
G_PER_CORE = N_GAUSS // N_CORES      # 2048
CHUNK = 128                          # gaussians per matmul chunk
N_CHUNKS = G_PER_CORE // CHUNK       # 16
ARG_W = 256                          # per-chunk arg width: 128 x | 128 y
OUT_W = 512                          # (c,y) free width of one accumulator

F32 = mybir.dt.float32
MM_DT = mybir.dt.float16             # block dtype: fp16 rounding of B is
# shared by num and den so it cancels in the ratio; colors carry an
# independent 2^-11 rounding which averages out over gaussians.
KARG = 11                            # arg-matmul contraction rows
N_WARM = 11                          # dense PE warmup matmuls (HAM gate)
ONE_TT = True                        # one bcast tensor_tensor vs 3 muls
BLK = 640                            # per-chunk fused block width
COEF_W = G_PER_CORE + ARG_W          # packed [coef | basis] columns


def build_program():
    """One SPMD Bass program; every core runs it on its gaussian slice."""
    nc = bacc.Bacc("TRN2", target_bir_lowering=False, debug=False,
                   num_devices=N_CORES)
    # [11, 2048+256] fp16: stationary coefficient rows (one chunk per 128
    # cols) packed with the 256 fixed basis cols so ONE DMA delivers both
    coefrhs = nc.dram_tensor("coefrhs", [KARG, COEF_W], MM_DT,
                             kind="ExternalInput")
    # [128, 16*384] fp16: colrep[p, c*384 + b*128 + y] = col[c*128+p, b]
    # (host-replicated so the color build is one broadcast multiply)
    colrep = nc.dram_tensor("colrep", [128, N_CHUNKS * 384], MM_DT,
                            kind="ExternalInput")
    # [128, 64] fp32 per-gaussian color scalars (fallback 3-mul path)
    opc = nc.dram_tensor("opc", [128, N_CHUNKS * 4], F32,
                         kind="ExternalInput")
    # two partial accumulators: [x, (den|r|g|b)*128+y] each
    out = nc.dram_tensor("out", [128, 2 * OUT_W], F32, kind="ExternalOutput")

    with tile.TileContext(nc) as tc:
        with tc.tile_pool(name="ins", bufs=1) as ins_pool, \
             tc.tile_pool(name="expp", bufs=1) as exp_pool, \
             tc.tile_pool(name="args", bufs=3, space="PSUM") as arg_pool, \
             tc.tile_pool(name="acc", bufs=1, space="PSUM") as acc_pool, \
             tc.tile_pool(name="outp", bufs=1) as out_pool:

            cr_t = ins_pool.tile([KARG, COEF_W], MM_DT)
            colrep_t = ins_pool.tile([128, N_CHUNKS, 3, 128], MM_DT)
            opc_t = ins_pool.tile([128, N_CHUNKS * 4], F32)

            # Warmup source memset on the otherwise-idle Vector engine so
            # the PE can start immediately; the critical coefrhs DMA goes
            # alone on the sync ring.  Only colrep quarter A (chunks 0-3)
            # shares the fabric with it — quarters B/C/D are gated behind a
            # tiny copy that depends on coefrhs, so their 1.2MB of traffic
            # cannot delay the matmul-critical 50KB.
            wsrc = ins_pool.tile([128, ARG_W], mybir.dt.bfloat16)
            nc.vector.memset(wsrc, 0.0)
            nc.sync.dma_start(out=cr_t, in_=coefrhs[:, :])
            Q = N_CHUNKS // 4
            gate_t = ins_pool.tile([KARG, 8], MM_DT)
            if ONE_TT:
                nc.scalar.dma_start(
                    out=colrep_t[:, 0:Q, :, :],
                    in_=colrep[:, 0:Q * 384],
                )
                nc.gpsimd.tensor_copy(gate_t, cr_t[:, 0:8])
                for qi in range(1, 4):
                    nc.gpsimd.dma_start(
                        out=colrep_t[:, qi * Q:(qi + 1) * Q, :, :],
                        in_=colrep[:, qi * Q * 384:(qi + 1) * Q * 384],
                    )
            else:
                nc.gpsimd.dma_start(out=opc_t, in_=opc[:, :])

            # fused per-chunk block [expx(128) | B(128) | colors(384)]
            t3 = exp_pool.tile([128, N_CHUNKS, BLK], MM_DT)
            accA = acc_pool.tile([128, OUT_W], F32)
            accB = acc_pool.tile([128, OUT_W], F32)

            # Dense PE warmup into accA (overwritten by main0's start=True):
            # HAM opens the clock gate to 8/8 (2.4GHz) only after a
            # fully-busy free-running 3413ns window; these run while the
            # input DMAs are in flight and hand off to the real stream.
            for _ in range(N_WARM):
                nc.tensor.matmul(accA[:, :ARG_W], wsrc[:, :CHUNK],
                                 wsrc[:, :], start=True, stop=True)

            rhs0 = G_PER_CORE                      # basis cols offset
            group_plan = [(0, 1), (1, 1), (2, 2), (4, 4), (8, 4), (12, 4)]

            def issue_args(g0c, width):
                args = arg_pool.tile([128, width * ARG_W], F32, tag="args")
                for k in range(width):
                    chunk = g0c + k
                    nc.tensor.matmul(
                        args[:, k * ARG_W:(k + 1) * ARG_W],
                        cr_t[:, chunk * CHUNK:(chunk + 1) * CHUNK],
                        cr_t[:, rhs0:rhs0 + ARG_W],
                        start=True, stop=True,
                    )
                nc.scalar.activation(
                    out=t3[:, g0c:g0c + width, 0:ARG_W],
                    in_=args[:, :width * ARG_W],
                    func=mybir.ActivationFunctionType.Exp,
                )
                for k in range(width):
                    chunk = g0c + k
                    # y half of the exp is B = op*expy (ln(op) in the
                    # arg); color blocks multiply the SAME rounded B so
                    # num/den rounding cancels.  Column order: [den|r|g|b]
                    if ONE_TT:
                        b3 = t3[:, chunk, 128:256].unsqueeze(1) \
                            .broadcast_to([128, 3, 128])
                        o3 = t3[:, chunk, 256:640].rearrange(
                            "p (c y) -> p c y", c=3)
                        nc.vector.tensor_tensor(
                            out=o3, in0=b3, in1=colrep_t[:, chunk, :, :],
                            op=mybir.AluOpType.mult,
                        )
                    else:
                        for c in range(3):
                            nc.vector.tensor_scalar_mul(
                                out=t3[:, chunk, 256 + c * 128:
                                       256 + (c + 1) * 128],
                                in0=t3[:, chunk, 128:256],
                                scalar1=opc_t[:, chunk * 4 + c:
                                              chunk * 4 + c + 1],
                            )

            def issue_main(chunk):
                acc = accA if chunk < 8 else accB
                nc.tensor.matmul(
                    acc[:, :],
                    t3[:, chunk, 0:128],
                    t3[:, chunk, 128:BLK],
                    start=(chunk % 8 == 0), stop=(chunk % 8 == 7),
                )

            # PE program order: args run 1-2 groups ahead of mains so the
            # engine queue always has work and never re-arms the throttle
            issue_args(0, 1)
            issue_args(1, 1)
            issue_args(2, 2)
            issue_main(0)
            issue_args(4, 4)
            for c in (1, 2, 3):
                issue_main(c)
            issue_args(8, 4)
            for c in (4, 5, 6, 7):
                issue_main(c)
            issue_args(12, 4)
            for c in range(8, 16):
                issue_main(c)

            out_t = out_pool.tile([128, 2 * OUT_W], F32)
            # accA final after main7: its copy+DMA overlap tail compute
            nc.scalar.copy(out=out_t[:, :OUT_W], in_=accA[:, :])
            nc.sync.dma_start(out=out[:, :OUT_W], in_=out_t[:, :OUT_W])
            # accB: split halves across engines/rings to shorten the tail
            nc.scalar.copy(out=out_t[:, OUT_W:OUT_W + 256],
                           in_=accB[:, :256])
            nc.vector.tensor_copy(out_t[:, OUT_W + 256:], accB[:, 256:])
            nc.sync.dma_start(out=out[:, OUT_W:OUT_W + 256],
                              in_=out_t[:, OUT_W:OUT_W + 256])
            nc.gpsimd.dma_start(out=out[:, OUT_W + 256:],
                                in_=out_t[:, OUT_W + 256:])

    nc.compile()
    return nc


_PROGRAM = None


def _get_program():
    global _PROGRAM
    if _PROGRAM is None:
        _PROGRAM = build_program()
    return _PROGRAM


def _quat2mat(q):
    q = q / np.linalg.norm(q)
    w, x, y, z = q
    return np.array([
        [1 - 2 * (y * y + z * z), 2 * (x * y - z * w), 2 * (x * z + y * w)],
        [2 * (x * y + z * w), 1 - 2 * (x * x + z * z), 2 * (y * z - x * w)],
        [2 * (x * z - y * w), 2 * (y * z + x * w), 1 - 2 * (x * x + y * y)],
    ])


def _hilo16(x):
    """Split x (f64) into fp16-representable hi+lo with hi+lo ~= x."""
    hi = np.asarray(x, dtype=np.float16)
    lo = (np.asarray(x, dtype=np.float64) - hi.astype(np.float64)) \
        .astype(np.float16)
    return hi, lo


def kernel(positions, colors, opacities, scales, qvec, tvec, tile_hw,
           chunk_gauss, _trace=False):
    positions = np.asarray(positions, dtype=np.float32)
    colors = np.asarray(colors, dtype=np.float32)
    opacities = np.asarray(opacities, dtype=np.float32)
    scales = np.asarray(scales, dtype=np.float32)
    qvec = np.asarray(qvec, dtype=np.float32)
    tvec = np.asarray(tvec, dtype=np.float32)
    tile_hw = int(tile_hw)
    chunk_gauss = int(chunk_gauss)
    n = positions.shape[0]
    assert n == N_GAUSS, f"expected {N_GAUSS} gaussians, got {n}"

    # ---- O(N) per-gaussian prep in float64 (rounds to the same f32 values
    # the reference computes, to well within the exp's own error budget) ----
    R = _quat2mat(qvec.astype(np.float64))
    cam = positions.astype(np.float64) @ R.T + tvec.astype(np.float64)
    ax = cam[:, 0] / cam[:, 2] * FX + CX          # [N] screen x center
    ay = cam[:, 1] / cam[:, 2] * FY + CY          # [N] screen y center
    var = scales[:, 0].astype(np.float64) ** 2
    s = -0.5 / var                                # [N] negative inv 2*var

    # centered coords keep the quadratic-expansion terms small (|u|<=64)
    dx = ax - CX
    dy = ay - CY
    op64 = opacities[:, 0].astype(np.float64)

    # K=11 stationary rows per gaussian (fp16), for
    #   arg_x = s*u^2 + (-2 s dx)*u + s*dx^2            (u = x - 64)
    #   arg_y = s*v^2 + (-2 s dy)*v + s*dy^2 + ln(op)   (v = y - 64)
    # The u^2 basis is split into b1 = fp16(u^2) (exact products against
    # hi/lo halves of s) and the residual b2 = u^2-b1 in {-1,0,1} handled
    # by a single-precision s row; s_lo is pre-scaled by 2^12 (and its
    # basis row by 2^-12) to stay inside fp16 normal range.
    s_hi = s.astype(np.float16)
    s_lo12 = ((s - s_hi.astype(np.float64)) * 4096.0).astype(np.float16)
    mx_hi, mx_lo = _hilo16(-2.0 * s * dx)
    my_hi, my_lo = _hilo16(-2.0 * s * dy)
    cx_hi, cx_lo = _hilo16(s * dx * dx)
    cy_hi, cy_lo = _hilo16(s * dy * dy + np.log(op64))
    coef_full = np.stack([s_hi, s_lo12, s_hi,
                          mx_hi, mx_lo, cx_hi, cx_lo,
                          my_hi, my_lo, cy_hi, cy_lo])   # [11, N] fp16

    u = np.arange(W, dtype=np.float64) - CX
    v = np.arange(H, dtype=np.float64) - CY
    b1u = (u * u).astype(np.float16).astype(np.float64)
    b2u = u * u - b1u
    b1v = (v * v).astype(np.float16).astype(np.float64)
    b2v = v * v - b1v
    zer = np.zeros(128)
    one = np.ones(128)
    rhs_rows = [
        np.concatenate([b1u, b1v]),                   # s_hi
        np.concatenate([b1u, b1v]) / 4096.0,          # s_lo12
        np.concatenate([b2u, b2v]),                   # s_hi (residual row)
        np.concatenate([u, zer]),                     # mx_hi
        np.concatenate([u, zer]),                     # mx_lo
        np.concatenate([one, zer]),                   # cx_hi
        np.concatenate([one, zer]),                   # cx_lo
        np.concatenate([zer, v]),                     # my_hi
        np.concatenate([zer, v]),                     # my_lo
        np.concatenate([zer, one]),                   # cy_hi
        np.concatenate([zer, one]),                   # cy_lo
    ]
    rhsxy = np.stack(rhs_rows).astype(np.float16)     # [11, 256]

    col16 = colors.astype(np.float16)                 # [N, 3]
    opc_full = np.concatenate(
        [colors, np.ones((n, 1), np.float32)], axis=1).astype(np.float32)

    # ---- shard gaussians across the 8 cores ----
    in_maps = []
    for core in range(N_CORES):
        g0 = core * G_PER_CORE
        g1 = g0 + G_PER_CORE
        coefrhs = np.concatenate(
            [coef_full[:, g0:g1], rhsxy], axis=1)     # [11, 2304]
        # colrep[p, chunk, c, y] = col16[g0 + chunk*128 + p, c]
        cc = col16[g0:g1].reshape(N_CHUNKS, CHUNK, 3)
        colrep_c = np.ascontiguousarray(
            np.broadcast_to(cc.transpose(1, 0, 2)[:, :, :, None],
                            (CHUNK, N_CHUNKS, 3, 128))
            .reshape(CHUNK, N_CHUNKS * 384))
        opc_c = opc_full[g0:g1].reshape(N_CHUNKS, CHUNK, 4)
        opc_c = np.ascontiguousarray(
            opc_c.transpose(1, 0, 2).reshape(CHUNK, N_CHUNKS * 4))
        in_maps.append({
            "coefrhs": np.ascontiguousarray(coefrhs),
            "colrep": colrep_c,
            "opc": opc_c,
        })

    nc = _get_program()
    res = run_bass_kernel_spmd(nc, in_maps, list(range(N_CORES)),
                               trace=_trace)

    # ---- host reduction: sum per-core partials, divide, reshape ----
    acc = np.zeros((128, 4, 128), dtype=np.float64)   # [x, (den|r|g|b), y]
    for core in range(N_CORES):
        o = res.results[core]["out"]
        acc += o[:, :OUT_W].reshape(128, 4, 128)
        acc += o[:, OUT_W:].reshape(128, 4, 128)

    num = acc[:, 1:4, :]                          # [x, c, y]
    n_chunks_ref = n // chunk_gauss
    den = acc[:, 0, :] + n_chunks_ref * EPS       # [x, y]
    img = num / den[:, None, :]                   # [x, c, y]
    img = img.transpose(2, 0, 1).reshape(H * W, 3)  # [p=(y,x), c]

    step = tile_hw * tile_hw
    t = (H * W) // step
    out = img.reshape(t, step, 3).transpose(0, 2, 1).reshape(
        t, 3, tile_hw, tile_hw)
    result = out.astype(np.float32)
    if _trace:
        return result, res
    return result


# revision 9
# speedup vs baseline: 1.1000x; 1.0598x over previous
"""Trainium2 Bass kernel for the isotropic-gaussian differentiable renderer.

Math: for pixel p=(x,y) and gaussian g:
    w[g,p] = op_g * exp(-0.5*((x-ax_g)^2+(y-ay_g)^2)/var_g)
    img[p,c] = (sum_g w[g,p]*col_gc) / (sum_g w[g,p] + n_chunks*EPS)

The isotropic RBF is separable: w = exp(sx) * B with
sx = s*(x-ax)^2, B = op*exp(s*(y-ay)^2), s = -0.5/var.  Per 128-gaussian
chunk:

  PE (fp16): arg[g, 0:128]=sx(g,x), arg[g,128:256]=sy(g,y)+ln(op) via a
             K=11 matmul against fixed basis rows.  fp16 stays exact
             because the quadratic basis u^2 is split into b1 (fp16-exact)
             + b2 (residual in {-1,0,1}) rows and every coefficient is a
             hi/lo fp16 pair (lo of the shared s row pre-scaled by 2^12 to
             stay in normal range); the catastrophically-cancelling
             quadratic matches fp64 to ~2e-5.
  ACT      : exp(arg) -> fp16 into fused per-chunk blocks
             [expx(128) | B(128) | colors(384)]; the y half lands as the
             den block B = op*expy directly (ln(op) is in the argument)
  DVE      : ONE broadcast tensor_tensor per chunk fills all 3 color
             blocks: B (stride-0 broadcast x3) * colrep (host-replicated
             [r*128|g*128|b*128] per gaussian).  One dispatch instead of
             three keeps the Vector engine far off the critical path.
  PE (fp16): acc += block[0:128]^T @ block[128:640] (fp32 PSUM
             accumulate); chunks 0-7 into accA, 8-15 into accB so accA's
             result DMA overlaps the tail of compute (host sums partials)

The PE is warmed with dense dummy matmuls from ~t=0.3us that connect
seamlessly into the real matmul stream: the HAM clock gate needs one
fully-busy 3413ns window before it opens to 2.4GHz, and any idle gap
re-arms the throttle.

Sharding: gaussians split 2048/core across 8 cores; every core accumulates
the full 128x128 image; host sums the 16 partials (2 per core), divides
num/den and reshapes to the reference's [4,3,64,64] tile layout.
"""
import numpy as np

import concourse.bacc as bacc
import concourse.tile as tile
from concourse import mybir
from concourse.bass_utils import run_bass_kernel_spmd

# Problem constants (hardcoded per harness contract)
N_GAUSS = 16384
H = 128
W = 128
FX = 128.0
FY = 128.0
CX = 64.0
CY = 64.0
EPS = 1e-8
N_CORES = 8
G_PER_CORE = N_GAUSS // N_CORES      # 2048
CHUNK = 128                          # gaussians per matmul chunk
N_CHUNKS = G_PER_CORE // CHUNK       # 16
ARG_W = 256                          # per-chunk arg width: 128 x | 128 y
OUT_W = 512                          # (c,y) free width of one accumulator

F32 = mybir.dt.float32
MM_DT = mybir.dt.float16             # block dtype: fp16 rounding of B is
# shared by num and den so it cancels in the ratio; colors carry an
# independent 2^-11 rounding which averages out over gaussians.
KARG = 11                            # arg-matmul contraction rows
N_WARM = 11                          # dense PE warmup matmuls (HAM gate)
ONE_TT = False                       # one bcast tensor_tensor vs 3 muls
# (the 1.5MB replicated-color DMA saturates HBM and delays the critical
# coefficient load by ~3us; three 163ns Vector muls are cheaper overall)
BLK = 640                            # per-chunk fused block width
COEF_W = G_PER_CORE + ARG_W          # packed [coef | basis] columns


def build_program():
    """One SPMD Bass program; every core runs it on its gaussian slice."""
    nc = bacc.Bacc("TRN2", target_bir_lowering=False, debug=False,
                   num_devices=N_CORES)
    # [11, 2048+256] fp16: stationary coefficient rows (one chunk per 128
    # cols) packed with the 256 fixed basis cols so ONE DMA delivers both
    coefrhs = nc.dram_tensor("coefrhs", [KARG, COEF_W], MM_DT,
                             kind="ExternalInput")
    # [128, 16*384] fp16: colrep[p, c*384 + b*128 + y] = col[c*128+p, b]
    # (host-replicated so the color build is one broadcast multiply)
    colrep = nc.dram_tensor("colrep", [128, N_CHUNKS * 384], MM_DT,
                            kind="ExternalInput")
    # [128, 64] fp32 per-gaussian color scalars (fallback 3-mul path)
    opc = nc.dram_tensor("opc", [128, N_CHUNKS * 4], F32,
                         kind="ExternalInput")
    # two partial accumulators: [x, (den|r|g|b)*128+y] each
    out = nc.dram_tensor("out", [128, 2 * OUT_W], F32, kind="ExternalOutput")

    with tile.TileContext(nc) as tc:
        with tc.tile_pool(name="ins", bufs=1) as ins_pool, \
             tc.tile_pool(name="expp", bufs=1) as exp_pool, \
             tc.tile_pool(name="args", bufs=3, space="PSUM") as arg_pool, \
             tc.tile_pool(name="acc", bufs=1, space="PSUM") as acc_pool, \
             tc.tile_pool(name="outp", bufs=1) as out_pool:

            cr_t = ins_pool.tile([KARG, COEF_W], MM_DT)
            colrep_t = ins_pool.tile([128, N_CHUNKS, 3, 128], MM_DT)
            opc_t = ins_pool.tile([128, N_CHUNKS * 4], F32)

            # Warmup source memset on the otherwise-idle Vector engine so
            # the PE can start immediately; the critical coefrhs DMA goes
            # alone on the sync ring.  Only colrep quarter A (chunks 0-3)
            # shares the fabric with it — quarters B/C/D are gated behind a
            # tiny copy that depends on coefrhs, so their 1.2MB of traffic
            # cannot delay the matmul-critical 50KB.
            wsrc = ins_pool.tile([128, ARG_W], mybir.dt.bfloat16)
            nc.vector.memset(wsrc, 0.0)
            nc.sync.dma_start(out=cr_t, in_=coefrhs[:, :])
            Q = N_CHUNKS // 4
            gate_t = ins_pool.tile([KARG, 8], MM_DT)
            if ONE_TT:
                nc.scalar.dma_start(
                    out=colrep_t[:, 0:Q, :, :],
                    in_=colrep[:, 0:Q * 384],
                )
                nc.gpsimd.tensor_copy(gate_t, cr_t[:, 0:8])
                for qi in range(1, 4):
                    nc.gpsimd.dma_start(
                        out=colrep_t[:, qi * Q:(qi + 1) * Q, :, :],
                        in_=colrep[:, qi * Q * 384:(qi + 1) * Q * 384],
                    )
            else:
                nc.gpsimd.dma_start(out=opc_t, in_=opc[:, :])

            # fused per-chunk block [expx(128) | B(128) | colors(384)]
            t3 = exp_pool.tile([128, N_CHUNKS, BLK], MM_DT)
            accA = acc_pool.tile([128, OUT_W], F32)
            accB = acc_pool.tile([128, OUT_W], F32)

            # Dense PE warmup into accA (overwritten by main0's start=True):
            # HAM opens the clock gate to 8/8 (2.4GHz) only after a
            # fully-busy free-running 3413ns window; these run while the
            # input DMAs are in flight and hand off to the real stream.
            for _ in range(N_WARM):
                nc.tensor.matmul(accA[:, :ARG_W], wsrc[:, :CHUNK],
                                 wsrc[:, :], start=True, stop=True)

            rhs0 = G_PER_CORE                      # basis cols offset
            group_plan = [(0, 1), (1, 1), (2, 2), (4, 4), (8, 4), (12, 4)]

            def issue_args(g0c, width):
                args = arg_pool.tile([128, width * ARG_W], F32, tag="args")
                for k in range(width):
                    chunk = g0c + k
                    nc.tensor.matmul(
                        args[:, k * ARG_W:(k + 1) * ARG_W],
                        cr_t[:, chunk * CHUNK:(chunk + 1) * CHUNK],
                        cr_t[:, rhs0:rhs0 + ARG_W],
                        start=True, stop=True,
                    )
                nc.scalar.activation(
                    out=t3[:, g0c:g0c + width, 0:ARG_W],
                    in_=args[:, :width * ARG_W],
                    func=mybir.ActivationFunctionType.Exp,
                )
                for k in range(width):
                    chunk = g0c + k
                    # y half of the exp is B = op*expy (ln(op) in the
                    # arg); color blocks multiply the SAME rounded B so
                    # num/den rounding cancels.  Column order: [den|r|g|b]
                    if ONE_TT:
                        b3 = t3[:, chunk, 128:256].unsqueeze(1) \
                            .broadcast_to([128, 3, 128])
                        o3 = t3[:, chunk, 256:640].rearrange(
                            "p (c y) -> p c y", c=3)
                        nc.vector.tensor_tensor(
                            out=o3, in0=b3, in1=colrep_t[:, chunk, :, :],
                            op=mybir.AluOpType.mult,
                        )
                    else:
                        for c in range(3):
                            nc.vector.tensor_scalar_mul(
                                out=t3[:, chunk, 256 + c * 128:
                                       256 + (c + 1) * 128],
                                in0=t3[:, chunk, 128:256],
                                scalar1=opc_t[:, chunk * 4 + c:
                                              chunk * 4 + c + 1],
                            )

            def issue_main(chunk):
                acc = accA if chunk < 8 else accB
                nc.tensor.matmul(
                    acc[:, :],
                    t3[:, chunk, 0:128],
                    t3[:, chunk, 128:BLK],
                    start=(chunk % 8 == 0), stop=(chunk % 8 == 7),
                )

            # PE program order: args run 1-2 groups ahead of mains so the
            # engine queue always has work and never re-arms the throttle
            issue_args(0, 1)
            issue_args(1, 1)
            issue_args(2, 2)
            issue_main(0)
            issue_args(4, 4)
            for c in (1, 2, 3):
                issue_main(c)
            issue_args(8, 4)
            for c in (4, 5, 6, 7):
                issue_main(c)
            issue_args(12, 4)
            for c in range(8, 16):
                issue_main(c)

            out_t = out_pool.tile([128, 2 * OUT_W], F32)
            # accA final after main7: its copy+DMA overlap tail compute
            nc.scalar.copy(out=out_t[:, :OUT_W], in_=accA[:, :])
            nc.sync.dma_start(out=out[:, :OUT_W], in_=out_t[:, :OUT_W])
            # accB: split halves across engines/rings to shorten the tail
            nc.scalar.copy(out=out_t[:, OUT_W:OUT_W + 256],
                           in_=accB[:, :256])
            nc.vector.tensor_copy(out_t[:, OUT_W + 256:], accB[:, 256:])
            nc.sync.dma_start(out=out[:, OUT_W:OUT_W + 256],
                              in_=out_t[:, OUT_W:OUT_W + 256])
            nc.gpsimd.dma_start(out=out[:, OUT_W + 256:],
                                in_=out_t[:, OUT_W + 256:])

    nc.compile()
    return nc


_PROGRAM = None


def _get_program():
    global _PROGRAM
    if _PROGRAM is None:
        _PROGRAM = build_program()
    return _PROGRAM


def _quat2mat(q):
    q = q / np.linalg.norm(q)
    w, x, y, z = q
    return np.array([
        [1 - 2 * (y * y + z * z), 2 * (x * y - z * w), 2 * (x * z + y * w)],
        [2 * (x * y + z * w), 1 - 2 * (x * x + z * z), 2 * (y * z - x * w)],
        [2 * (x * z - y * w), 2 * (y * z + x * w), 1 - 2 * (x * x + y * y)],
    ])


def _hilo16(x):
    """Split x (f64) into fp16-representable hi+lo with hi+lo ~= x."""
    hi = np.asarray(x, dtype=np.float16)
    lo = (np.asarray(x, dtype=np.float64) - hi.astype(np.float64)) \
        .astype(np.float16)
    return hi, lo


def kernel(positions, colors, opacities, scales, qvec, tvec, tile_hw,
           chunk_gauss, _trace=False):
    positions = np.asarray(positions, dtype=np.float32)
    colors = np.asarray(colors, dtype=np.float32)
    opacities = np.asarray(opacities, dtype=np.float32)
    scales = np.asarray(scales, dtype=np.float32)
    qvec = np.asarray(qvec, dtype=np.float32)
    tvec = np.asarray(tvec, dtype=np.float32)
    tile_hw = int(tile_hw)
    chunk_gauss = int(chunk_gauss)
    n = positions.shape[0]
    assert n == N_GAUSS, f"expected {N_GAUSS} gaussians, got {n}"

    # ---- O(N) per-gaussian prep in float64 (rounds to the same f32 values
    # the reference computes, to well within the exp's own error budget) ----
    R = _quat2mat(qvec.astype(np.float64))
    cam = positions.astype(np.float64) @ R.T + tvec.astype(np.float64)
    ax = cam[:, 0] / cam[:, 2] * FX + CX          # [N] screen x center
    ay = cam[:, 1] / cam[:, 2] * FY + CY          # [N] screen y center
    var = scales[:, 0].astype(np.float64) ** 2
    s = -0.5 / var                                # [N] negative inv 2*var

    # centered coords keep the quadratic-expansion terms small (|u|<=64)
    dx = ax - CX
    dy = ay - CY
    op64 = opacities[:, 0].astype(np.float64)

    # K=11 stationary rows per gaussian (fp16), for
    #   arg_x = s*u^2 + (-2 s dx)*u + s*dx^2            (u = x - 64)
    #   arg_y = s*v^2 + (-2 s dy)*v + s*dy^2 + ln(op)   (v = y - 64)
    # The u^2 basis is split into b1 = fp16(u^2) (exact products against
    # hi/lo halves of s) and the residual b2 = u^2-b1 in {-1,0,1} handled
    # by a single-precision s row; s_lo is pre-scaled by 2^12 (and its
    # basis row by 2^-12) to stay inside fp16 normal range.
    s_hi = s.astype(np.float16)
    s_lo12 = ((s - s_hi.astype(np.float64)) * 4096.0).astype(np.float16)
    mx_hi, mx_lo = _hilo16(-2.0 * s * dx)
    my_hi, my_lo = _hilo16(-2.0 * s * dy)
    cx_hi, cx_lo = _hilo16(s * dx * dx)
    cy_hi, cy_lo = _hilo16(s * dy * dy + np.log(op64))
    coef_full = np.stack([s_hi, s_lo12, s_hi,
                          mx_hi, mx_lo, cx_hi, cx_lo,
                          my_hi, my_lo, cy_hi, cy_lo])   # [11, N] fp16

    u = np.arange(W, dtype=np.float64) - CX
    v = np.arange(H, dtype=np.float64) - CY
    b1u = (u * u).astype(np.float16).astype(np.float64)
    b2u = u * u - b1u
    b1v = (v * v).astype(np.float16).astype(np.float64)
    b2v = v * v - b1v
    zer = np.zeros(128)
    one = np.ones(128)
    rhs_rows = [
        np.concatenate([b1u, b1v]),                   # s_hi
        np.concatenate([b1u, b1v]) / 4096.0,          # s_lo12
        np.concatenate([b2u, b2v]),                   # s_hi (residual row)
        np.concatenate([u, zer]),                     # mx_hi
        np.concatenate([u, zer]),                     # mx_lo
        np.concatenate([one, zer]),                   # cx_hi
        np.concatenate([one, zer]),                   # cx_lo
        np.concatenate([zer, v]),                     # my_hi
        np.concatenate([zer, v]),                     # my_lo
        np.concatenate([zer, one]),                   # cy_hi
        np.concatenate([zer, one]),                   # cy_lo
    ]
    rhsxy = np.stack(rhs_rows).astype(np.float16)     # [11, 256]

    col16 = colors.astype(np.float16)                 # [N, 3]
    opc_full = np.concatenate(
        [colors, np.ones((n, 1), np.float32)], axis=1).astype(np.float32)

    # ---- shard gaussians across the 8 cores ----
    in_maps = []
    for core in range(N_CORES):
        g0 = core * G_PER_CORE
        g1 = g0 + G_PER_CORE
        coefrhs = np.concatenate(
            [coef_full[:, g0:g1], rhsxy], axis=1)     # [11, 2304]
        # colrep[p, chunk, c, y] = col16[g0 + chunk*128 + p, c]
        cc = col16[g0:g1].reshape(N_CHUNKS, CHUNK, 3)
        colrep_c = np.ascontiguousarray(
            np.broadcast_to(cc.transpose(1, 0, 2)[:, :, :, None],
                            (CHUNK, N_CHUNKS, 3, 128))
            .reshape(CHUNK, N_CHUNKS * 384))
        opc_c = opc_full[g0:g1].reshape(N_CHUNKS, CHUNK, 4)
        opc_c = np.ascontiguousarray(
            opc_c.transpose(1, 0, 2).reshape(CHUNK, N_CHUNKS * 4))
        in_maps.append({
            "coefrhs": np.ascontiguousarray(coefrhs),
            "colrep": colrep_c,
            "opc": opc_c,
        })

    nc = _get_program()
    res = run_bass_kernel_spmd(nc, in_maps, list(range(N_CORES)),
                               trace=_trace)

    # ---- host reduction: sum per-core partials, divide, reshape ----
    acc = np.zeros((128, 4, 128), dtype=np.float64)   # [x, (den|r|g|b), y]
    for core in range(N_CORES):
        o = res.results[core]["out"]
        acc += o[:, :OUT_W].reshape(128, 4, 128)
        acc += o[:, OUT_W:].reshape(128, 4, 128)

    num = acc[:, 1:4, :]                          # [x, c, y]
    n_chunks_ref = n // chunk_gauss
    den = acc[:, 0, :] + n_chunks_ref * EPS       # [x, y]
    img = num / den[:, None, :]                   # [x, c, y]
    img = img.transpose(2, 0, 1).reshape(H * W, 3)  # [p=(y,x), c]

    step = tile_hw * tile_hw
    t = (H * W) // step
    out = img.reshape(t, step, 3).transpose(0, 2, 1).reshape(
        t, 3, tile_hw, tile_hw)
    result = out.astype(np.float32)
    if _trace:
        return result, res
    return result


# revision 13
# speedup vs baseline: 1.1084x; 1.0076x over previous
"""Trainium2 Bass kernel for the isotropic-gaussian differentiable renderer.

Math: for pixel p=(x,y) and gaussian g:
    w[g,p] = op_g * exp(-0.5*((x-ax_g)^2+(y-ay_g)^2)/var_g)
    img[p,c] = (sum_g w[g,p]*col_gc) / (sum_g w[g,p] + n_chunks*EPS)

The isotropic RBF is separable: w = exp(sx) * B with
sx = s*(x-ax)^2, B = op*exp(s*(y-ay)^2), s = -0.5/var.  Per 128-gaussian
chunk:

  PE (fp16): arg[g, 0:128]=sx(g,x), arg[g,128:256]=sy(g,y)+ln(op) via a
             K=11 matmul against fixed basis rows.  fp16 stays exact
             because the quadratic basis u^2 is split into b1 (fp16-exact)
             + b2 (residual in {-1,0,1}) rows and every coefficient is a
             hi/lo fp16 pair (lo of the shared s row pre-scaled by 2^12 to
             stay in normal range); the catastrophically-cancelling
             quadratic matches fp64 to ~2e-5.
  ACT      : exp(arg) -> fp16 into fused per-chunk blocks
             [expx(128) | B(128) | colors(384)]; the y half lands as the
             den block B = op*expy directly (ln(op) is in the argument)
  DVE      : ONE broadcast tensor_tensor per chunk fills all 3 color
             blocks: B (stride-0 broadcast x3) * colrep (host-replicated
             [r*128|g*128|b*128] per gaussian).  One dispatch instead of
             three keeps the Vector engine far off the critical path.
  PE (fp16): acc += block[0:128]^T @ block[128:640] (fp32 PSUM
             accumulate); chunks 0-7 into accA, 8-15 into accB so accA's
             result DMA overlaps the tail of compute (host sums partials)

The PE is warmed with dense dummy matmuls from ~t=0.3us that connect
seamlessly into the real matmul stream: the HAM clock gate needs one
fully-busy 3413ns window before it opens to 2.4GHz, and any idle gap
re-arms the throttle.

Sharding: gaussians split 2048/core across 8 cores; every core accumulates
the full 128x128 image; host sums the 16 partials (2 per core), divides
num/den and reshapes to the reference's [4,3,64,64] tile layout.
"""
import numpy as np

import concourse.bacc as bacc
import concourse.tile as tile
from concourse import mybir
from concourse.bass_utils import run_bass_kernel_spmd

# Problem constants (hardcoded per harness contract)
N_GAUSS = 16384
H = 128
W = 128
FX = 128.0
FY = 128.0
CX = 64.0
CY = 64.0
EPS = 1e-8
N_CORES = 8
G_PER_CORE = N_GAUSS // N_CORES      # 2048
CHUNK = 128                          # gaussians per matmul chunk
N_CHUNKS = G_PER_CORE // CHUNK       # 16
ARG_W = 256                          # per-chunk arg width: 128 x | 128 y
OUT_W = 512                          # (c,y) free width of one accumulator

F32 = mybir.dt.float32
MM_DT = mybir.dt.float16             # coef/basis dtype for the arg matmul
BLK_DT = mybir.dt.float8e4           # block dtype (e4m3): expx rounding is
# shared by num and den so it cancels in the ratio; B/color rounding is
# independent per gaussian (~3% rms) and averages out over the hundreds of
# gaussians covering each pixel.  e4m3 enables DoubleRow matmuls: two
# 128-gaussian chunks contract per main matmul at 2 rows/cycle.
KARG = 11                            # arg-matmul contraction rows
N_WARM = 11                          # dense PE warmup matmuls (HAM gate)
ONE_TT = False                       # one bcast tensor_tensor vs 3 muls
# (the 1.5MB replicated-color DMA saturates HBM and delays the critical
# coefficient load by ~3us; three 163ns Vector muls are cheaper overall)
BLK = 640                            # per-chunk fused block width
COEF_W = G_PER_CORE + ARG_W          # packed [coef | basis] columns


def build_program():
    """One SPMD Bass program; every core runs it on its gaussian slice."""
    nc = bacc.Bacc("TRN2", target_bir_lowering=False, debug=False,
                   num_devices=N_CORES)
    # [11, 2048+256] fp16: stationary coefficient rows (one chunk per 128
    # cols) packed with the 256 fixed basis cols so ONE DMA delivers both
    coefrhs = nc.dram_tensor("coefrhs", [KARG, COEF_W], MM_DT,
                             kind="ExternalInput")
    # [128, 16*384] fp16: colrep[p, c*384 + b*128 + y] = col[c*128+p, b]
    # (host-replicated so the color build is one broadcast multiply)
    colrep = nc.dram_tensor("colrep", [128, N_CHUNKS * 384], MM_DT,
                            kind="ExternalInput")
    # [128, 64] fp32 per-gaussian color scalars (fallback 3-mul path)
    opc = nc.dram_tensor("opc", [128, N_CHUNKS * 4], F32,
                         kind="ExternalInput")
    # two partial accumulators: [x, (den|r|g|b)*128+y] each
    out = nc.dram_tensor("out", [128, 2 * OUT_W], F32, kind="ExternalOutput")

    with tile.TileContext(nc) as tc:
        with tc.tile_pool(name="ins", bufs=1) as ins_pool, \
             tc.tile_pool(name="expp", bufs=1) as exp_pool, \
             tc.tile_pool(name="args", bufs=3, space="PSUM") as arg_pool, \
             tc.tile_pool(name="acc", bufs=1, space="PSUM") as acc_pool, \
             tc.tile_pool(name="outp", bufs=1) as out_pool:

            cr_t = ins_pool.tile([KARG, COEF_W], MM_DT)
            colrep_t = ins_pool.tile([128, N_CHUNKS, 3, 128], MM_DT)
            opc_t = ins_pool.tile([128, N_CHUNKS * 4], F32)

            # Warmup source memset on the otherwise-idle Vector engine so
            # the PE can start immediately; the critical coefrhs DMA goes
            # alone on the sync ring.  Only colrep quarter A (chunks 0-3)
            # shares the fabric with it — quarters B/C/D are gated behind a
            # tiny copy that depends on coefrhs, so their 1.2MB of traffic
            # cannot delay the matmul-critical 50KB.
            wsrc = ins_pool.tile([128, ARG_W], mybir.dt.bfloat16)
            nc.vector.memset(wsrc, 0.0)
            nc.sync.dma_start(out=cr_t, in_=coefrhs[:, :])
            Q = N_CHUNKS // 4
            gate_t = ins_pool.tile([KARG, 8], MM_DT)
            if ONE_TT:
                nc.scalar.dma_start(
                    out=colrep_t[:, 0:Q, :, :],
                    in_=colrep[:, 0:Q * 384],
                )
                nc.gpsimd.tensor_copy(gate_t, cr_t[:, 0:8])
                for qi in range(1, 4):
                    nc.gpsimd.dma_start(
                        out=colrep_t[:, qi * Q:(qi + 1) * Q, :, :],
                        in_=colrep[:, qi * Q * 384:(qi + 1) * Q * 384],
                    )
            else:
                nc.gpsimd.dma_start(out=opc_t, in_=opc[:, :])

            # fused per-chunk block [expx(128) | B(128) | colors(384)]
            t3 = exp_pool.tile([128, N_CHUNKS, BLK], BLK_DT)
            accA = acc_pool.tile([128, OUT_W], F32)
            accB = acc_pool.tile([128, OUT_W], F32)

            # Dense PE warmup into accA (overwritten by main0's start=True):
            # HAM opens the clock gate to 8/8 (2.4GHz) only after a
            # fully-busy free-running 3413ns window; these run while the
            # input DMAs are in flight and hand off to the real stream.
            for _ in range(N_WARM):
                nc.tensor.matmul(accA[:, :ARG_W], wsrc[:, :CHUNK],
                                 wsrc[:, :], start=True, stop=True)

            rhs0 = G_PER_CORE                      # basis cols offset
            group_plan = [(0, 1), (1, 1), (2, 2), (4, 4), (8, 4), (12, 4)]

            def issue_args(g0c, width):
                args = arg_pool.tile([128, width * ARG_W], F32, tag="args")
                for k in range(width):
                    chunk = g0c + k
                    nc.tensor.matmul(
                        args[:, k * ARG_W:(k + 1) * ARG_W],
                        cr_t[:, chunk * CHUNK:(chunk + 1) * CHUNK],
                        cr_t[:, rhs0:rhs0 + ARG_W],
                        start=True, stop=True,
                    )
                nc.scalar.activation(
                    out=t3[:, g0c:g0c + width, 0:ARG_W],
                    in_=args[:, :width * ARG_W],
                    func=mybir.ActivationFunctionType.Exp,
                )
                for k in range(width):
                    chunk = g0c + k
                    # y half of the exp is B = op*expy (ln(op) in the
                    # arg); color blocks multiply the SAME rounded B so
                    # num/den rounding cancels.  Column order: [den|r|g|b]
                    if ONE_TT:
                        b3 = t3[:, chunk, 128:256].unsqueeze(1) \
                            .broadcast_to([128, 3, 128])
                        o3 = t3[:, chunk, 256:640].rearrange(
                            "p (c y) -> p c y", c=3)
                        nc.vector.tensor_tensor(
                            out=o3, in0=b3, in1=colrep_t[:, chunk, :, :],
                            op=mybir.AluOpType.mult,
                        )
                    else:
                        for c in range(3):
                            nc.vector.tensor_scalar_mul(
                                out=t3[:, chunk, 256 + c * 128:
                                       256 + (c + 1) * 128],
                                in0=t3[:, chunk, 128:256],
                                scalar1=opc_t[:, chunk * 4 + c:
                                              chunk * 4 + c + 1],
                            )

            def issue_main(pair):
                # DoubleRow: chunks 2p and 2p+1 contract in one matmul —
                # lhsT [128, (2,128)] and rhs [128, (2,512)] pair-strided
                # views of the fused blocks, out accumulates [128, 512]
                c = 2 * pair
                acc = accA if pair < 4 else accB
                nc.tensor.matmul(
                    acc[:, :],
                    t3[:, c:c + 2, 0:128],
                    t3[:, c:c + 2, 128:BLK],
                    start=(pair % 4 == 0), stop=(pair % 4 == 3),
                    perf_mode=mybir.MatmulPerfMode.DoubleRow,
                )

            # PE program order: args run 1-2 groups ahead of mains so the
            # engine queue always has work and never re-arms the throttle
            issue_args(0, 1)
            issue_args(1, 1)
            issue_args(2, 2)
            issue_main(0)
            issue_args(4, 4)
            issue_main(1)
            issue_args(8, 4)
            for p in (2, 3):
                issue_main(p)
            issue_args(12, 4)
            for p in range(4, 8):
                issue_main(p)

            out_t = out_pool.tile([128, 2 * OUT_W], F32)
            # accA final after pair 3: its copy+DMA overlap tail compute
            nc.scalar.copy(out=out_t[:, :OUT_W], in_=accA[:, :])
            nc.sync.dma_start(out=out[:, :OUT_W], in_=out_t[:, :OUT_W])
            # accB: halves copied on scalar+vector in parallel; scalar
            # triggers its own half's DMA (no cross-engine sem hop), sync
            # (idle) triggers the other
            nc.scalar.copy(out=out_t[:, OUT_W:OUT_W + 256],
                           in_=accB[:, :256])
            nc.scalar.dma_start(out=out[:, OUT_W:OUT_W + 256],
                                in_=out_t[:, OUT_W:OUT_W + 256])
            nc.vector.tensor_copy(out_t[:, OUT_W + 256:], accB[:, 256:])
            nc.sync.dma_start(out=out[:, OUT_W + 256:],
                              in_=out_t[:, OUT_W + 256:])

    nc.compile()
    return nc


_PROGRAM = None


def _get_program():
    global _PROGRAM
    if _PROGRAM is None:
        _PROGRAM = build_program()
    return _PROGRAM


def _quat2mat(q):
    q = q / np.linalg.norm(q)
    w, x, y, z = q
    return np.array([
        [1 - 2 * (y * y + z * z), 2 * (x * y - z * w), 2 * (x * z + y * w)],
        [2 * (x * y + z * w), 1 - 2 * (x * x + z * z), 2 * (y * z - x * w)],
        [2 * (x * z - y * w), 2 * (y * z + x * w), 1 - 2 * (x * x + y * y)],
    ])


def _hilo16(x):
    """Split x (f64) into fp16-representable hi+lo with hi+lo ~= x."""
    hi = np.asarray(x, dtype=np.float16)
    lo = (np.asarray(x, dtype=np.float64) - hi.astype(np.float64)) \
        .astype(np.float16)
    return hi, lo


def kernel(positions, colors, opacities, scales, qvec, tvec, tile_hw,
           chunk_gauss, _trace=False):
    positions = np.asarray(positions, dtype=np.float32)
    colors = np.asarray(colors, dtype=np.float32)
    opacities = np.asarray(opacities, dtype=np.float32)
    scales = np.asarray(scales, dtype=np.float32)
    qvec = np.asarray(qvec, dtype=np.float32)
    tvec = np.asarray(tvec, dtype=np.float32)
    tile_hw = int(tile_hw)
    chunk_gauss = int(chunk_gauss)
    n = positions.shape[0]
    assert n == N_GAUSS, f"expected {N_GAUSS} gaussians, got {n}"

    # ---- O(N) per-gaussian prep in float64 (rounds to the same f32 values
    # the reference computes, to well within the exp's own error budget) ----
    R = _quat2mat(qvec.astype(np.float64))
    cam = positions.astype(np.float64) @ R.T + tvec.astype(np.float64)
    ax = cam[:, 0] / cam[:, 2] * FX + CX          # [N] screen x center
    ay = cam[:, 1] / cam[:, 2] * FY + CY          # [N] screen y center
    var = scales[:, 0].astype(np.float64) ** 2
    s = -0.5 / var                                # [N] negative inv 2*var

    # centered coords keep the quadratic-expansion terms small (|u|<=64)
    dx = ax - CX
    dy = ay - CY
    op64 = opacities[:, 0].astype(np.float64)

    # K=11 stationary rows per gaussian (fp16), for
    #   arg_x = s*u^2 + (-2 s dx)*u + s*dx^2            (u = x - 64)
    #   arg_y = s*v^2 + (-2 s dy)*v + s*dy^2 + ln(op)   (v = y - 64)
    # The u^2 basis is split into b1 = fp16(u^2) (exact products against
    # hi/lo halves of s) and the residual b2 = u^2-b1 in {-1,0,1} handled
    # by a single-precision s row; s_lo is pre-scaled by 2^12 (and its
    # basis row by 2^-12) to stay inside fp16 normal range.
    s_hi = s.astype(np.float16)
    s_lo12 = ((s - s_hi.astype(np.float64)) * 4096.0).astype(np.float16)
    mx_hi, mx_lo = _hilo16(-2.0 * s * dx)
    my_hi, my_lo = _hilo16(-2.0 * s * dy)
    cx_hi, cx_lo = _hilo16(s * dx * dx)
    cy_hi, cy_lo = _hilo16(s * dy * dy + np.log(op64))
    coef_full = np.stack([s_hi, s_lo12, s_hi,
                          mx_hi, mx_lo, cx_hi, cx_lo,
                          my_hi, my_lo, cy_hi, cy_lo])   # [11, N] fp16

    u = np.arange(W, dtype=np.float64) - CX
    v = np.arange(H, dtype=np.float64) - CY
    b1u = (u * u).astype(np.float16).astype(np.float64)
    b2u = u * u - b1u
    b1v = (v * v).astype(np.float16).astype(np.float64)
    b2v = v * v - b1v
    zer = np.zeros(128)
    one = np.ones(128)
    rhs_rows = [
        np.concatenate([b1u, b1v]),                   # s_hi
        np.concatenate([b1u, b1v]) / 4096.0,          # s_lo12
        np.concatenate([b2u, b2v]),                   # s_hi (residual row)
        np.concatenate([u, zer]),                     # mx_hi
        np.concatenate([u, zer]),                     # mx_lo
        np.concatenate([one, zer]),                   # cx_hi
        np.concatenate([one, zer]),                   # cx_lo
        np.concatenate([zer, v]),                     # my_hi
        np.concatenate([zer, v]),                     # my_lo
        np.concatenate([zer, one]),                   # cy_hi
        np.concatenate([zer, one]),                   # cy_lo
    ]
    rhsxy = np.stack(rhs_rows).astype(np.float16)     # [11, 256]

    col16 = colors.astype(np.float16)                 # [N, 3]
    opc_full = np.concatenate(
        [colors, np.ones((n, 1), np.float32)], axis=1).astype(np.float32)

    # ---- shard gaussians across the 8 cores ----
    in_maps = []
    for core in range(N_CORES):
        g0 = core * G_PER_CORE
        g1 = g0 + G_PER_CORE
        coefrhs = np.concatenate(
            [coef_full[:, g0:g1], rhsxy], axis=1)     # [11, 2304]
        # colrep[p, chunk, c, y] = col16[g0 + chunk*128 + p, c]
        cc = col16[g0:g1].reshape(N_CHUNKS, CHUNK, 3)
        colrep_c = np.ascontiguousarray(
            np.broadcast_to(cc.transpose(1, 0, 2)[:, :, :, None],
                            (CHUNK, N_CHUNKS, 3, 128))
            .reshape(CHUNK, N_CHUNKS * 384))
        opc_c = opc_full[g0:g1].reshape(N_CHUNKS, CHUNK, 4)
        opc_c = np.ascontiguousarray(
            opc_c.transpose(1, 0, 2).reshape(CHUNK, N_CHUNKS * 4))
        in_maps.append({
            "coefrhs": np.ascontiguousarray(coefrhs),
            "colrep": colrep_c,
            "opc": opc_c,
        })

    nc = _get_program()
    res = run_bass_kernel_spmd(nc, in_maps, list(range(N_CORES)),
                               trace=_trace)

    # ---- host reduction: sum per-core partials, divide, reshape ----
    acc = np.zeros((128, 4, 128), dtype=np.float64)   # [x, (den|r|g|b), y]
    for core in range(N_CORES):
        o = res.results[core]["out"]
        acc += o[:, :OUT_W].reshape(128, 4, 128)
        acc += o[:, OUT_W:].reshape(128, 4, 128)

    num = acc[:, 1:4, :]                          # [x, c, y]
    n_chunks_ref = n // chunk_gauss
    den = acc[:, 0, :] + n_chunks_ref * EPS       # [x, y]
    img = num / den[:, None, :]                   # [x, c, y]
    img = img.transpose(2, 0, 1).reshape(H * W, 3)  # [p=(y,x), c]

    step = tile_hw * tile_hw
    t = (H * W) // step
    out = img.reshape(t, step, 3).transpose(0, 2, 1).reshape(
        t, 3, tile_hw, tile_hw)
    result = out.astype(np.float32)
    if _trace:
        return result, res
    return result


# revision 18
# speedup vs baseline: 1.1111x; 1.0025x over previous
"""Trainium2 Bass kernel for the isotropic-gaussian differentiable renderer.

Math: for pixel p=(x,y) and gaussian g:
    w[g,p] = op_g * exp(-0.5*((x-ax_g)^2+(y-ay_g)^2)/var_g)
    img[p,c] = (sum_g w[g,p]*col_gc) / (sum_g w[g,p] + n_chunks*EPS)

The isotropic RBF is separable: w = exp(sx) * B with
sx = s*(x-ax)^2, B = op*exp(s*(y-ay)^2), s = -0.5/var.  Per 128-gaussian
chunk:

  PE (fp16): arg[g, 0:128]=sx(g,x), arg[g,128:256]=sy(g,y)+ln(op) via a
             K=11 matmul against fixed basis rows.  fp16 stays exact
             because the quadratic basis u^2 is split into b1 (fp16-exact)
             + b2 (residual in {-1,0,1}) rows and every coefficient is a
             hi/lo fp16 pair (lo of the shared s row pre-scaled by 2^12 to
             stay in normal range); the catastrophically-cancelling
             quadratic matches fp64 to ~2e-5.
  ACT      : exp(arg) -> fp16 into fused per-chunk blocks
             [expx(128) | B(128) | colors(384)]; the y half lands as the
             den block B = op*expy directly (ln(op) is in the argument)
  DVE      : ONE broadcast tensor_tensor per chunk fills all 3 color
             blocks: B (stride-0 broadcast x3) * colrep (host-replicated
             [r*128|g*128|b*128] per gaussian).  One dispatch instead of
             three keeps the Vector engine far off the critical path.
  PE (fp16): acc += block[0:128]^T @ block[128:640] (fp32 PSUM
             accumulate); chunks 0-7 into accA, 8-15 into accB so accA's
             result DMA overlaps the tail of compute (host sums partials)

The PE is warmed with dense dummy matmuls from ~t=0.3us that connect
seamlessly into the real matmul stream: the HAM clock gate needs one
fully-busy 3413ns window before it opens to 2.4GHz, and any idle gap
re-arms the throttle.

Sharding: gaussians split 2048/core across 8 cores; every core accumulates
the full 128x128 image; host sums the 16 partials (2 per core), divides
num/den and reshapes to the reference's [4,3,64,64] tile layout.
"""
import numpy as np

import concourse.bacc as bacc
import concourse.tile as tile
from concourse import mybir
from concourse.bass_utils import run_bass_kernel_spmd

# Problem constants (hardcoded per harness contract)
N_GAUSS = 16384
H = 128
W = 128
FX = 128.0
FY = 128.0
CX = 64.0
CY = 64.0
EPS = 1e-8
N_CORES = 8
G_PER_CORE = N_GAUSS // N_CORES      # 2048
CHUNK = 128                          # gaussians per matmul chunk
N_CHUNKS = G_PER_CORE // CHUNK       # 16
ARG_W = 256                          # per-chunk arg width: 128 x | 128 y
OUT_W = 512                          # (c,y) free width of one accumulator

F32 = mybir.dt.float32
MM_DT = mybir.dt.float16             # coef/basis dtype for the arg matmul
BLK_DT = mybir.dt.float8e4           # block dtype (e4m3): expx rounding is
# shared by num and den so it cancels in the ratio; B/color rounding is
# independent per gaussian (~3% rms) and averages out over the hundreds of
# gaussians covering each pixel.  e4m3 enables DoubleRow matmuls: two
# 128-gaussian chunks contract per main matmul at 2 rows/cycle.
KARG = 13                            # arg-matmul contraction rows
N_WARM = 11                          # dense PE warmup matmuls (HAM gate)
ONE_TT = False                       # one bcast tensor_tensor vs 3 muls
# (the 1.5MB replicated-color DMA saturates HBM and delays the critical
# coefficient load by ~3us; Vector muls are cheaper overall)
BLK = 640                            # per-chunk fused block width
AW = 384                             # arg cols: x | y_den | y_r — the red
# block rides the exp (ln(op*col_r) folded into its constant row), so the
# Vector engine only builds 2 color blocks per chunk
PSLOT = 512                          # PSUM cols per chunk arg (matmul
# outputs must not cross 2KB PSUM bank boundaries, so 384 pads to 512)
COEF_W = G_PER_CORE + AW             # packed [coef | basis] columns


def build_program():
    """One SPMD Bass program; every core runs it on its gaussian slice."""
    nc = bacc.Bacc("TRN2", target_bir_lowering=False, debug=False,
                   num_devices=N_CORES)
    # [11, 2048+256] fp16: stationary coefficient rows (one chunk per 128
    # cols) packed with the 256 fixed basis cols so ONE DMA delivers both
    coefrhs = nc.dram_tensor("coefrhs", [KARG, COEF_W], MM_DT,
                             kind="ExternalInput")
    # [128, 16*384] fp16: colrep[p, c*384 + b*128 + y] = col[c*128+p, b]
    # (host-replicated so the color build is one broadcast multiply)
    colrep = nc.dram_tensor("colrep", [128, N_CHUNKS * 384], MM_DT,
                            kind="ExternalInput")
    # [128, 64] fp32 per-gaussian color scalars (fallback 3-mul path)
    opc = nc.dram_tensor("opc", [128, N_CHUNKS * 4], F32,
                         kind="ExternalInput")
    # two partial accumulators: [x, (den|r|g|b)*128+y] each
    out = nc.dram_tensor("out", [128, 2 * OUT_W], F32, kind="ExternalOutput")

    with tile.TileContext(nc) as tc:
        with tc.tile_pool(name="ins", bufs=1) as ins_pool, \
             tc.tile_pool(name="expp", bufs=1) as exp_pool, \
             tc.tile_pool(name="args", bufs=2, space="PSUM") as arg_pool, \
             tc.tile_pool(name="acc", bufs=1, space="PSUM") as acc_pool, \
             tc.tile_pool(name="outp", bufs=1) as out_pool:

            cr_t = ins_pool.tile([KARG, COEF_W], MM_DT)
            colrep_t = ins_pool.tile([128, N_CHUNKS, 3, 128], MM_DT)
            opc_t = ins_pool.tile([128, N_CHUNKS * 4], F32)

            # Warmup source memset on the otherwise-idle Vector engine so
            # the PE can start immediately; the critical coefrhs DMA goes
            # alone on the sync ring.  Only colrep quarter A (chunks 0-3)
            # shares the fabric with it — quarters B/C/D are gated behind a
            # tiny copy that depends on coefrhs, so their 1.2MB of traffic
            # cannot delay the matmul-critical 50KB.
            wsrc = ins_pool.tile([128, ARG_W], mybir.dt.bfloat16)
            nc.vector.memset(wsrc, 0.0)
            nc.sync.dma_start(out=cr_t, in_=coefrhs[:, :])
            Q = N_CHUNKS // 4
            gate_t = ins_pool.tile([KARG, 8], MM_DT)
            if ONE_TT:
                nc.scalar.dma_start(
                    out=colrep_t[:, 0:Q, :, :],
                    in_=colrep[:, 0:Q * 384],
                )
                nc.gpsimd.tensor_copy(gate_t, cr_t[:, 0:8])
                for qi in range(1, 4):
                    nc.gpsimd.dma_start(
                        out=colrep_t[:, qi * Q:(qi + 1) * Q, :, :],
                        in_=colrep[:, qi * Q * 384:(qi + 1) * Q * 384],
                    )
            else:
                nc.gpsimd.dma_start(out=opc_t, in_=opc[:, :])

            # fused per-chunk block [expx(128) | B(128) | colors(384)]
            t3 = exp_pool.tile([128, N_CHUNKS, BLK], BLK_DT)
            accA = acc_pool.tile([128, OUT_W], F32)
            accB = acc_pool.tile([128, OUT_W], F32)

            # Dense PE warmup into accA (overwritten by main0's start=True):
            # HAM opens the clock gate to 8/8 (2.4GHz) only after a
            # fully-busy free-running 3413ns window; these run while the
            # input DMAs are in flight and hand off to the real stream.
            for _ in range(N_WARM):
                nc.tensor.matmul(accA[:, :ARG_W], wsrc[:, :CHUNK],
                                 wsrc[:, :], start=True, stop=True)

            rhs0 = G_PER_CORE                      # basis cols offset
            group_plan = [(0, 1), (1, 1), (2, 2), (4, 3), (7, 3), (10, 3),
                          (13, 3)]

            def issue_args(g0c, width):
                args = arg_pool.tile([128, width, PSLOT], F32, tag="args")
                for k in range(width):
                    chunk = g0c + k
                    nc.tensor.matmul(
                        args[:, k, 0:AW],
                        cr_t[:, chunk * CHUNK:(chunk + 1) * CHUNK],
                        cr_t[:, rhs0:rhs0 + AW],
                        start=True, stop=True,
                    )
                # exp writes [expx | B | colR] per chunk: the y half is
                # B = op*expy and the third block is colR = op*col_r*expy
                # (their ln() offsets ride the arg's constant rows)
                nc.scalar.activation(
                    out=t3[:, g0c:g0c + width, 0:AW],
                    in_=args[:, :, 0:AW],
                    func=mybir.ActivationFunctionType.Exp,
                )
                for k in range(width):
                    chunk = g0c + k
                    # remaining color blocks multiply the SAME rounded B
                    # so num/den rounding cancels.  Columns: [den|r|g|b]
                    for c in (1, 2):
                        nc.vector.tensor_scalar_mul(
                            out=t3[:, chunk, 256 + c * 128:
                                   256 + (c + 1) * 128],
                            in0=t3[:, chunk, 128:256],
                            scalar1=opc_t[:, chunk * 4 + c:
                                          chunk * 4 + c + 1],
                        )

            def issue_main(pair):
                # DoubleRow: chunks 2p and 2p+1 contract in one matmul —
                # lhsT [128, (2,128)] and rhs [128, (2,512)] pair-strided
                # views of the fused blocks, out accumulates [128, 512]
                c = 2 * pair
                acc = accA if pair < 4 else accB
                nc.tensor.matmul(
                    acc[:, :],
                    t3[:, c:c + 2, 0:128],
                    t3[:, c:c + 2, 128:BLK],
                    start=(pair % 4 == 0), stop=(pair % 4 == 3),
                    perf_mode=mybir.MatmulPerfMode.DoubleRow,
                )

            # PE program order: args run 1-2 groups ahead of mains so the
            # engine queue always has work and never re-arms the throttle
            issue_args(0, 1)
            issue_args(1, 1)
            issue_args(2, 2)
            issue_main(0)
            issue_args(4, 3)
            issue_main(1)
            issue_args(7, 3)
            issue_main(2)
            issue_args(10, 3)
            issue_main(3)
            issue_args(13, 3)
            for p in range(4, 8):
                issue_main(p)

            out_t = out_pool.tile([128, 2 * OUT_W], F32)
            # accA final after pair 3: its copy+DMA overlap tail compute
            nc.scalar.copy(out=out_t[:, :OUT_W], in_=accA[:, :])
            nc.sync.dma_start(out=out[:, :OUT_W], in_=out_t[:, :OUT_W])
            # accB: halves copied on scalar+vector in parallel; scalar
            # triggers its own half's DMA (no cross-engine sem hop), sync
            # (idle) triggers the other
            nc.scalar.copy(out=out_t[:, OUT_W:OUT_W + 256],
                           in_=accB[:, :256])
            nc.scalar.dma_start(out=out[:, OUT_W:OUT_W + 256],
                                in_=out_t[:, OUT_W:OUT_W + 256])
            nc.vector.tensor_copy(out_t[:, OUT_W + 256:], accB[:, 256:])
            nc.sync.dma_start(out=out[:, OUT_W + 256:],
                              in_=out_t[:, OUT_W + 256:])

    nc.compile()
    return nc


_PROGRAM = None


def _get_program():
    global _PROGRAM
    if _PROGRAM is None:
        _PROGRAM = build_program()
    return _PROGRAM


def _quat2mat(q):
    q = q / np.linalg.norm(q)
    w, x, y, z = q
    return np.array([
        [1 - 2 * (y * y + z * z), 2 * (x * y - z * w), 2 * (x * z + y * w)],
        [2 * (x * y + z * w), 1 - 2 * (x * x + z * z), 2 * (y * z - x * w)],
        [2 * (x * z - y * w), 2 * (y * z + x * w), 1 - 2 * (x * x + y * y)],
    ])


def _hilo16(x):
    """Split x (f64) into fp16-representable hi+lo with hi+lo ~= x."""
    hi = np.asarray(x, dtype=np.float16)
    lo = (np.asarray(x, dtype=np.float64) - hi.astype(np.float64)) \
        .astype(np.float16)
    return hi, lo


def kernel(positions, colors, opacities, scales, qvec, tvec, tile_hw,
           chunk_gauss, _trace=False):
    positions = np.asarray(positions, dtype=np.float32)
    colors = np.asarray(colors, dtype=np.float32)
    opacities = np.asarray(opacities, dtype=np.float32)
    scales = np.asarray(scales, dtype=np.float32)
    qvec = np.asarray(qvec, dtype=np.float32)
    tvec = np.asarray(tvec, dtype=np.float32)
    tile_hw = int(tile_hw)
    chunk_gauss = int(chunk_gauss)
    n = positions.shape[0]
    assert n == N_GAUSS, f"expected {N_GAUSS} gaussians, got {n}"

    # ---- O(N) per-gaussian prep in float64 (rounds to the same f32 values
    # the reference computes, to well within the exp's own error budget) ----
    R = _quat2mat(qvec.astype(np.float64))
    cam = positions.astype(np.float64) @ R.T + tvec.astype(np.float64)
    ax = cam[:, 0] / cam[:, 2] * FX + CX          # [N] screen x center
    ay = cam[:, 1] / cam[:, 2] * FY + CY          # [N] screen y center
    var = scales[:, 0].astype(np.float64) ** 2
    s = -0.5 / var                                # [N] negative inv 2*var

    # centered coords keep the quadratic-expansion terms small (|u|<=64)
    dx = ax - CX
    dy = ay - CY
    op64 = opacities[:, 0].astype(np.float64)

    # K=13 stationary rows per gaussian (fp16), for args [x | y | y_r]:
    #   arg_x  = s*u^2 + (-2 s dx)*u + s*dx^2                (u = x - 64)
    #   arg_y  = s*v^2 + (-2 s dy)*v + s*dy^2 + ln(op)       (v = y - 64)
    #   arg_yr = s*v^2 + (-2 s dy)*v + s*dy^2 + ln(op*col_r)
    # The u^2 basis is split into b1 = fp16(u^2) (exact products against
    # hi/lo halves of s) and the residual b2 = u^2-b1 in {-1,0,1} handled
    # by a single-precision s row; s_lo is pre-scaled by 2^12 (and its
    # basis row by 2^-12) to stay inside fp16 normal range.
    s_hi = s.astype(np.float16)
    s_lo12 = ((s - s_hi.astype(np.float64)) * 4096.0).astype(np.float16)
    mx_hi, mx_lo = _hilo16(-2.0 * s * dx)
    my_hi, my_lo = _hilo16(-2.0 * s * dy)
    cx_hi, cx_lo = _hilo16(s * dx * dx)
    lnop = np.log(op64)
    cy_hi, cy_lo = _hilo16(s * dy * dy + lnop)
    col64 = colors.astype(np.float64)
    lnopr = np.maximum(lnop + np.log(np.maximum(col64[:, 0], 1e-30)), -40.0)
    cyr_hi, cyr_lo = _hilo16(s * dy * dy + lnopr)
    coef_full = np.stack([s_hi, s_lo12, s_hi,
                          mx_hi, mx_lo, cx_hi, cx_lo,
                          my_hi, my_lo, cy_hi, cy_lo,
                          cyr_hi, cyr_lo])            # [13, N] fp16

    u = np.arange(W, dtype=np.float64) - CX
    v = np.arange(H, dtype=np.float64) - CY
    b1u = (u * u).astype(np.float16).astype(np.float64)
    b2u = u * u - b1u
    b1v = (v * v).astype(np.float16).astype(np.float64)
    b2v = v * v - b1v
    zer = np.zeros(128)
    one = np.ones(128)
    rhs_rows = [
        np.concatenate([b1u, b1v, b1v]),              # s_hi
        np.concatenate([b1u, b1v, b1v]) / 4096.0,     # s_lo12
        np.concatenate([b2u, b2v, b2v]),              # s_hi (residual row)
        np.concatenate([u, zer, zer]),                # mx_hi
        np.concatenate([u, zer, zer]),                # mx_lo
        np.concatenate([one, zer, zer]),              # cx_hi
        np.concatenate([one, zer, zer]),              # cx_lo
        np.concatenate([zer, v, v]),                  # my_hi
        np.concatenate([zer, v, v]),                  # my_lo
        np.concatenate([zer, one, zer]),              # cy_hi
        np.concatenate([zer, one, zer]),              # cy_lo
        np.concatenate([zer, zer, one]),              # cyr_hi
        np.concatenate([zer, zer, one]),              # cyr_lo
    ]
    rhsxy = np.stack(rhs_rows).astype(np.float16)     # [13, 384]

    col16 = colors.astype(np.float16)                 # [N, 3]
    opc_full = np.concatenate(
        [colors, np.ones((n, 1), np.float32)], axis=1).astype(np.float32)

    # ---- shard gaussians across the 8 cores ----
    in_maps = []
    for core in range(N_CORES):
        g0 = core * G_PER_CORE
        g1 = g0 + G_PER_CORE
        coefrhs = np.concatenate(
            [coef_full[:, g0:g1], rhsxy], axis=1)     # [11, 2304]
        # colrep[p, chunk, c, y] = col16[g0 + chunk*128 + p, c]
        cc = col16[g0:g1].reshape(N_CHUNKS, CHUNK, 3)
        colrep_c = np.ascontiguousarray(
            np.broadcast_to(cc.transpose(1, 0, 2)[:, :, :, None],
                            (CHUNK, N_CHUNKS, 3, 128))
            .reshape(CHUNK, N_CHUNKS * 384))
        opc_c = opc_full[g0:g1].reshape(N_CHUNKS, CHUNK, 4)
        opc_c = np.ascontiguousarray(
            opc_c.transpose(1, 0, 2).reshape(CHUNK, N_CHUNKS * 4))
        in_maps.append({
            "coefrhs": np.ascontiguousarray(coefrhs),
            "colrep": colrep_c,
            "opc": opc_c,
        })

    nc = _get_program()
    res = run_bass_kernel_spmd(nc, in_maps, list(range(N_CORES)),
                               trace=_trace)

    # ---- host reduction: sum per-core partials, divide, reshape ----
    acc = np.zeros((128, 4, 128), dtype=np.float64)   # [x, (den|r|g|b), y]
    for core in range(N_CORES):
        o = res.results[core]["out"]
        acc += o[:, :OUT_W].reshape(128, 4, 128)
        acc += o[:, OUT_W:].reshape(128, 4, 128)

    num = acc[:, 1:4, :]                          # [x, c, y]
    n_chunks_ref = n // chunk_gauss
    den = acc[:, 0, :] + n_chunks_ref * EPS       # [x, y]
    img = num / den[:, None, :]                   # [x, c, y]
    img = img.transpose(2, 0, 1).reshape(H * W, 3)  # [p=(y,x), c]

    step = tile_hw * tile_hw
    t = (H * W) // step
    out = img.reshape(t, step, 3).transpose(0, 2, 1).reshape(
        t, 3, tile_hw, tile_hw)
    result = out.astype(np.float32)
    if _trace:
        return result, res
    return result


# revision 26
# speedup vs baseline: 1.1343x; 1.0209x over previous
"""Trainium2 Bass kernel for the isotropic-gaussian differentiable renderer.

Math: for pixel p=(x,y) and gaussian g:
    w[g,p] = op_g * exp(-0.5*((x-ax_g)^2+(y-ay_g)^2)/var_g)
    img[p,c] = (sum_g w[g,p]*col_gc) / (sum_g w[g,p] + n_chunks*EPS)

The isotropic RBF is separable: w = exp(sx) * B with
sx = s*(x-ax)^2, B = op*exp(s*(y-ay)^2), s = -0.5/var.  Per 128-gaussian
chunk:

  PE (fp16): arg[g, 0:128]=sx(g,x), arg[g,128:256]=sy(g,y)+ln(op) via a
             K=11 matmul against fixed basis rows.  fp16 stays exact
             because the quadratic basis u^2 is split into b1 (fp16-exact)
             + b2 (residual in {-1,0,1}) rows and every coefficient is a
             hi/lo fp16 pair (lo of the shared s row pre-scaled by 2^12 to
             stay in normal range); the catastrophically-cancelling
             quadratic matches fp64 to ~2e-5.
  ACT      : exp(arg) -> fp16 into fused per-chunk blocks
             [expx(128) | B(128) | colors(384)]; the y half lands as the
             den block B = op*expy directly (ln(op) is in the argument)
  DVE      : ONE broadcast tensor_tensor per chunk fills all 3 color
             blocks: B (stride-0 broadcast x3) * colrep (host-replicated
             [r*128|g*128|b*128] per gaussian).  One dispatch instead of
             three keeps the Vector engine far off the critical path.
  PE (fp16): acc += block[0:128]^T @ block[128:640] (fp32 PSUM
             accumulate); chunks 0-7 into accA, 8-15 into accB so accA's
             result DMA overlaps the tail of compute (host sums partials)

The PE is warmed with dense dummy matmuls from ~t=0.3us that connect
seamlessly into the real matmul stream: the HAM clock gate needs one
fully-busy 3413ns window before it opens to 2.4GHz, and any idle gap
re-arms the throttle.

Sharding: gaussians split 2048/core across 8 cores; every core accumulates
the full 128x128 image; host sums the 16 partials (2 per core), divides
num/den and reshapes to the reference's [4,3,64,64] tile layout.
"""
import numpy as np

import concourse.bacc as bacc
import concourse.tile as tile
from concourse import mybir
from concourse.bass_utils import run_bass_kernel_spmd

# Problem constants (hardcoded per harness contract)
N_GAUSS = 16384
H = 128
W = 128
FX = 128.0
FY = 128.0
CX = 64.0
CY = 64.0
EPS = 1e-8
N_CORES = 8
G_PER_CORE = N_GAUSS // N_CORES      # 2048
CHUNK = 128                          # gaussians per matmul chunk
N_CHUNKS = G_PER_CORE // CHUNK       # 16
ARG_W = 256                          # per-chunk arg width: 128 x | 128 y
OUT_W = 512                          # (c,y) free width of one accumulator

F32 = mybir.dt.float32
MM_DT = mybir.dt.float16             # coef/basis dtype for the arg matmul
BLK_DT = mybir.dt.float8e4           # block dtype (e4m3): expx rounding is
# shared by num and den so it cancels in the ratio; B/color rounding is
# independent per gaussian (~3% rms) and averages out over the hundreds of
# gaussians covering each pixel.  e4m3 enables DoubleRow matmuls: two
# 128-gaussian chunks contract per main matmul at 2 rows/cycle.
KARG = 13                            # arg-matmul contraction rows
N_WARM = 8                           # dense PE warmup matmuls (HAM gate)
ONE_TT = False                       # one bcast tensor_tensor vs 3 muls
# (the 1.5MB replicated-color DMA saturates HBM and delays the critical
# coefficient load by ~3us; Vector muls are cheaper overall)
BLK = 640                            # per-chunk fused block width
AW = 384                             # arg cols: x | y_den | y_r — the red
# block rides the exp (ln(op*col_r) folded into its constant row), so the
# Vector engine only builds 2 color blocks per chunk
PSLOT = 512                          # PSUM cols per chunk arg (matmul
# outputs must not cross 2KB PSUM bank boundaries, so 384 pads to 512)
COEF_W = G_PER_CORE + AW             # packed [coef | basis] columns


def build_program():
    """One SPMD Bass program; every core runs it on its gaussian slice."""
    nc = bacc.Bacc("TRN2", target_bir_lowering=False, debug=False,
                   num_devices=N_CORES)
    # [11, 2048+256] fp16: stationary coefficient rows (one chunk per 128
    # cols) packed with the 256 fixed basis cols so ONE DMA delivers both
    coefrhs = nc.dram_tensor("coefrhs", [KARG, COEF_W], MM_DT,
                             kind="ExternalInput")
    # [128, 16*384] fp16: colrep[p, c*384 + b*128 + y] = col[c*128+p, b]
    # (host-replicated so the color build is one broadcast multiply)
    colrep = nc.dram_tensor("colrep", [128, N_CHUNKS * 384], MM_DT,
                            kind="ExternalInput")
    # [128, 64] fp32 per-gaussian color scalars (fallback 3-mul path)
    opc = nc.dram_tensor("opc", [128, N_CHUNKS * 4], F32,
                         kind="ExternalInput")
    # two partial accumulators: [x, (den|r|g|b)*128+y] each
    out = nc.dram_tensor("out", [128, 2 * OUT_W], F32, kind="ExternalOutput")

    with tile.TileContext(nc) as tc:
        with tc.tile_pool(name="ins", bufs=1) as ins_pool, \
             tc.tile_pool(name="expp", bufs=1) as exp_pool, \
             tc.tile_pool(name="args", bufs=2, space="PSUM") as arg_pool, \
             tc.tile_pool(name="acc", bufs=1, space="PSUM") as acc_pool, \
             tc.tile_pool(name="outp", bufs=1) as out_pool:

            cr_t = ins_pool.tile([KARG, COEF_W], MM_DT)
            colrep_t = ins_pool.tile([128, N_CHUNKS, 3, 128], MM_DT)
            opc_t = ins_pool.tile([128, N_CHUNKS * 4], F32)

            # Warmup source memset on the otherwise-idle Vector engine so
            # the PE can start immediately; the critical coefrhs DMA goes
            # alone on the sync ring.  Only colrep quarter A (chunks 0-3)
            # shares the fabric with it — quarters B/C/D are gated behind a
            # tiny copy that depends on coefrhs, so their 1.2MB of traffic
            # cannot delay the matmul-critical 50KB.
            # coefrhs layout is [basis(384) | coefs(2048)]: the first DMA
            # slice carries the basis plus chunks 0-3 so the arg matmuls
            # start ~0.8us earlier; the rest follows on the same ring.
            # opc rides the scalar ring (queued before its table load).
            wsrc = ins_pool.tile([128, ARG_W], mybir.dt.bfloat16)
            nc.vector.memset(wsrc, 0.0)
            SLICE1 = AW + 4 * CHUNK
            nc.sync.dma_start(out=cr_t[:, :SLICE1], in_=coefrhs[:, :SLICE1])
            nc.sync.dma_start(out=cr_t[:, SLICE1:], in_=coefrhs[:, SLICE1:])
            nc.scalar.dma_start(out=opc_t, in_=opc[:, :])

            # fused per-chunk block [expx(128) | B(128) | colors(384)]
            t3 = exp_pool.tile([128, N_CHUNKS, BLK], BLK_DT)
            accA = acc_pool.tile([128, OUT_W], F32)
            accB = acc_pool.tile([128, OUT_W], F32)

            # Dense PE warmup into accA (overwritten by main0's start=True):
            # HAM opens the clock gate to 8/8 (2.4GHz) only after a
            # fully-busy free-running 3413ns window; these run while the
            # input DMAs are in flight and hand off to the real stream.
            for _ in range(N_WARM):
                nc.tensor.matmul(accA[:, :ARG_W], wsrc[:, :CHUNK],
                                 wsrc[:, :], start=True, stop=True)

            rhs0 = 0                               # basis cols offset
            coef0 = AW                             # coef cols offset
            group_plan = [(0, 1), (1, 1), (2, 2), (4, 3), (7, 3), (10, 3),
                          (13, 3)]

            def issue_args(g0c, width):
                args = arg_pool.tile([128, width, PSLOT], F32, tag="args")
                for k in range(width):
                    chunk = g0c + k
                    nc.tensor.matmul(
                        args[:, k, 0:AW],
                        cr_t[:, coef0 + chunk * CHUNK:
                             coef0 + (chunk + 1) * CHUNK],
                        cr_t[:, rhs0:rhs0 + AW],
                        start=True, stop=True,
                    )
                # exp writes [expx | B | colR] per chunk: the y half is
                # B = op*expy and the third block is colR = op*col_r*expy
                # (their ln() offsets ride the arg's constant rows)
                nc.scalar.activation(
                    out=t3[:, g0c:g0c + width, 0:AW],
                    in_=args[:, :, 0:AW],
                    func=mybir.ActivationFunctionType.Exp,
                )
                for k in range(width):
                    chunk = g0c + k
                    # remaining color blocks multiply the SAME rounded B
                    # so num/den rounding cancels.  Columns: [den|r|g|b]
                    for c in (1, 2):
                        nc.vector.tensor_scalar_mul(
                            out=t3[:, chunk, 256 + c * 128:
                                   256 + (c + 1) * 128],
                            in0=t3[:, chunk, 128:256],
                            scalar1=opc_t[:, chunk * 4 + c:
                                          chunk * 4 + c + 1],
                        )

            def issue_main(pair):
                # DoubleRow: chunks 2p and 2p+1 contract in one matmul —
                # lhsT [128, (2,128)] and rhs [128, (2,512)] pair-strided
                # views of the fused blocks, out accumulates [128, 512]
                c = 2 * pair
                acc = accA if pair < 4 else accB
                nc.tensor.matmul(
                    acc[:, :],
                    t3[:, c:c + 2, 0:128],
                    t3[:, c:c + 2, 128:BLK],
                    start=(pair % 4 == 0), stop=(pair % 4 == 3),
                    perf_mode=mybir.MatmulPerfMode.DoubleRow,
                )

            # PE program order: args run 1-2 groups ahead of mains so the
            # engine queue always has work and never re-arms the throttle
            issue_args(0, 1)
            issue_args(1, 1)
            issue_args(2, 2)
            issue_main(0)
            issue_args(4, 3)
            issue_main(1)
            issue_args(7, 3)
            issue_main(2)
            issue_args(10, 3)
            issue_main(3)
            issue_args(13, 3)
            for p in range(4, 8):
                issue_main(p)

            out_t = out_pool.tile([128, 2 * OUT_W], F32)
            # accA final after pair 3: its copy+DMA overlap tail compute
            nc.scalar.copy(out=out_t[:, :OUT_W], in_=accA[:, :])
            nc.sync.dma_start(out=out[:, :OUT_W], in_=out_t[:, :OUT_W])
            # accB: halves copied on scalar+vector in parallel; scalar
            # triggers its own half's DMA (no cross-engine sem hop), sync
            # (idle) triggers the other
            nc.scalar.copy(out=out_t[:, OUT_W:OUT_W + 256],
                           in_=accB[:, :256])
            nc.scalar.dma_start(out=out[:, OUT_W:OUT_W + 256],
                                in_=out_t[:, OUT_W:OUT_W + 256])
            nc.vector.tensor_copy(out_t[:, OUT_W + 256:], accB[:, 256:])
            nc.sync.dma_start(out=out[:, OUT_W + 256:],
                              in_=out_t[:, OUT_W + 256:])

    nc.compile()
    return nc


_PROGRAM = None


def _get_program():
    global _PROGRAM
    if _PROGRAM is None:
        _PROGRAM = build_program()
    return _PROGRAM


def _quat2mat(q):
    q = q / np.linalg.norm(q)
    w, x, y, z = q
    return np.array([
        [1 - 2 * (y * y + z * z), 2 * (x * y - z * w), 2 * (x * z + y * w)],
        [2 * (x * y + z * w), 1 - 2 * (x * x + z * z), 2 * (y * z - x * w)],
        [2 * (x * z - y * w), 2 * (y * z + x * w), 1 - 2 * (x * x + y * y)],
    ])


def _hilo16(x):
    """Split x (f64) into fp16-representable hi+lo with hi+lo ~= x."""
    hi = np.asarray(x, dtype=np.float16)
    lo = (np.asarray(x, dtype=np.float64) - hi.astype(np.float64)) \
        .astype(np.float16)
    return hi, lo


def kernel(positions, colors, opacities, scales, qvec, tvec, tile_hw,
           chunk_gauss, _trace=False):
    positions = np.asarray(positions, dtype=np.float32)
    colors = np.asarray(colors, dtype=np.float32)
    opacities = np.asarray(opacities, dtype=np.float32)
    scales = np.asarray(scales, dtype=np.float32)
    qvec = np.asarray(qvec, dtype=np.float32)
    tvec = np.asarray(tvec, dtype=np.float32)
    tile_hw = int(tile_hw)
    chunk_gauss = int(chunk_gauss)
    n = positions.shape[0]
    assert n == N_GAUSS, f"expected {N_GAUSS} gaussians, got {n}"

    # ---- O(N) per-gaussian prep in float64 (rounds to the same f32 values
    # the reference computes, to well within the exp's own error budget) ----
    R = _quat2mat(qvec.astype(np.float64))
    cam = positions.astype(np.float64) @ R.T + tvec.astype(np.float64)
    ax = cam[:, 0] / cam[:, 2] * FX + CX          # [N] screen x center
    ay = cam[:, 1] / cam[:, 2] * FY + CY          # [N] screen y center
    var = scales[:, 0].astype(np.float64) ** 2
    s = -0.5 / var                                # [N] negative inv 2*var

    # centered coords keep the quadratic-expansion terms small (|u|<=64)
    dx = ax - CX
    dy = ay - CY
    op64 = opacities[:, 0].astype(np.float64)

    # K=13 stationary rows per gaussian (fp16), for args [x | y | y_r]:
    #   arg_x  = s*u^2 + (-2 s dx)*u + s*dx^2                (u = x - 64)
    #   arg_y  = s*v^2 + (-2 s dy)*v + s*dy^2 + ln(op)       (v = y - 64)
    #   arg_yr = s*v^2 + (-2 s dy)*v + s*dy^2 + ln(op*col_r)
    # The u^2 basis is split into b1 = fp16(u^2) (exact products against
    # hi/lo halves of s) and the residual b2 = u^2-b1 in {-1,0,1} handled
    # by a single-precision s row; s_lo is pre-scaled by 2^12 (and its
    # basis row by 2^-12) to stay inside fp16 normal range.
    s_hi = s.astype(np.float16)
    s_lo12 = ((s - s_hi.astype(np.float64)) * 4096.0).astype(np.float16)
    mx_hi, mx_lo = _hilo16(-2.0 * s * dx)
    my_hi, my_lo = _hilo16(-2.0 * s * dy)
    cx_hi, cx_lo = _hilo16(s * dx * dx)
    lnop = np.log(op64)
    cy_hi, cy_lo = _hilo16(s * dy * dy + lnop)
    col64 = colors.astype(np.float64)
    lnopr = np.maximum(lnop + np.log(np.maximum(col64[:, 0], 1e-30)), -40.0)
    cyr_hi, cyr_lo = _hilo16(s * dy * dy + lnopr)
    coef_full = np.stack([s_hi, s_lo12, s_hi,
                          mx_hi, mx_lo, cx_hi, cx_lo,
                          my_hi, my_lo, cy_hi, cy_lo,
                          cyr_hi, cyr_lo])            # [13, N] fp16

    u = np.arange(W, dtype=np.float64) - CX
    v = np.arange(H, dtype=np.float64) - CY
    b1u = (u * u).astype(np.float16).astype(np.float64)
    b2u = u * u - b1u
    b1v = (v * v).astype(np.float16).astype(np.float64)
    b2v = v * v - b1v
    zer = np.zeros(128)
    one = np.ones(128)
    rhs_rows = [
        np.concatenate([b1u, b1v, b1v]),              # s_hi
        np.concatenate([b1u, b1v, b1v]) / 4096.0,     # s_lo12
        np.concatenate([b2u, b2v, b2v]),              # s_hi (residual row)
        np.concatenate([u, zer, zer]),                # mx_hi
        np.concatenate([u, zer, zer]),                # mx_lo
        np.concatenate([one, zer, zer]),              # cx_hi
        np.concatenate([one, zer, zer]),              # cx_lo
        np.concatenate([zer, v, v]),                  # my_hi
        np.concatenate([zer, v, v]),                  # my_lo
        np.concatenate([zer, one, zer]),              # cy_hi
        np.concatenate([zer, one, zer]),              # cy_lo
        np.concatenate([zer, zer, one]),              # cyr_hi
        np.concatenate([zer, zer, one]),              # cyr_lo
    ]
    rhsxy = np.stack(rhs_rows).astype(np.float16)     # [13, 384]

    col16 = colors.astype(np.float16)                 # [N, 3]
    opc_full = np.concatenate(
        [colors, np.ones((n, 1), np.float32)], axis=1).astype(np.float32)

    # ---- shard gaussians across the 8 cores ----
    in_maps = []
    for core in range(N_CORES):
        g0 = core * G_PER_CORE
        g1 = g0 + G_PER_CORE
        coefrhs = np.concatenate(
            [rhsxy, coef_full[:, g0:g1]], axis=1)     # [13, 384+2048]
        # colrep[p, chunk, c, y] = col16[g0 + chunk*128 + p, c]
        cc = col16[g0:g1].reshape(N_CHUNKS, CHUNK, 3)
        colrep_c = np.ascontiguousarray(
            np.broadcast_to(cc.transpose(1, 0, 2)[:, :, :, None],
                            (CHUNK, N_CHUNKS, 3, 128))
            .reshape(CHUNK, N_CHUNKS * 384))
        opc_c = opc_full[g0:g1].reshape(N_CHUNKS, CHUNK, 4)
        opc_c = np.ascontiguousarray(
            opc_c.transpose(1, 0, 2).reshape(CHUNK, N_CHUNKS * 4))
        in_maps.append({
            "coefrhs": np.ascontiguousarray(coefrhs),
            "colrep": colrep_c,
            "opc": opc_c,
        })

    nc = _get_program()
    res = run_bass_kernel_spmd(nc, in_maps, list(range(N_CORES)),
                               trace=_trace)

    # ---- host reduction: sum per-core partials, divide, reshape ----
    acc = np.zeros((128, 4, 128), dtype=np.float64)   # [x, (den|r|g|b), y]
    for core in range(N_CORES):
        o = res.results[core]["out"]
        acc += o[:, :OUT_W].reshape(128, 4, 128)
        acc += o[:, OUT_W:].reshape(128, 4, 128)

    num = acc[:, 1:4, :]                          # [x, c, y]
    n_chunks_ref = n // chunk_gauss
    den = acc[:, 0, :] + n_chunks_ref * EPS       # [x, y]
    img = num / den[:, None, :]                   # [x, c, y]
    img = img.transpose(2, 0, 1).reshape(H * W, 3)  # [p=(y,x), c]

    step = tile_hw * tile_hw
    t = (H * W) // step
    out = img.reshape(t, step, 3).transpose(0, 2, 1).reshape(
        t, 3, tile_hw, tile_hw)
    result = out.astype(np.float32)
    if _trace:
        return result, res
    return result


# revision 32
# speedup vs baseline: 1.1568x; 1.0198x over previous
"""Trainium2 Bass kernel for the isotropic-gaussian differentiable renderer.

Math: for pixel p=(x,y) and gaussian g:
    w[g,p] = op_g * exp(-0.5*((x-ax_g)^2+(y-ay_g)^2)/var_g)
    img[p,c] = (sum_g w[g,p]*col_gc) / (sum_g w[g,p] + n_chunks*EPS)

The isotropic RBF is separable: w = exp(sx) * B with
sx = s*(x-ax)^2, B = op*exp(s*(y-ay)^2), s = -0.5/var.  Per 128-gaussian
chunk:

  PE (fp16): arg[g, 0:128]=sx(g,x), arg[g,128:256]=sy(g,y)+ln(op) via a
             K=11 matmul against fixed basis rows.  fp16 stays exact
             because the quadratic basis u^2 is split into b1 (fp16-exact)
             + b2 (residual in {-1,0,1}) rows and every coefficient is a
             hi/lo fp16 pair (lo of the shared s row pre-scaled by 2^12 to
             stay in normal range); the catastrophically-cancelling
             quadratic matches fp64 to ~2e-5.
  ACT      : exp(arg) -> fp16 into fused per-chunk blocks
             [expx(128) | B(128) | colors(384)]; the y half lands as the
             den block B = op*expy directly (ln(op) is in the argument)
  DVE      : ONE broadcast tensor_tensor per chunk fills all 3 color
             blocks: B (stride-0 broadcast x3) * colrep (host-replicated
             [r*128|g*128|b*128] per gaussian).  One dispatch instead of
             three keeps the Vector engine far off the critical path.
  PE (fp16): acc += block[0:128]^T @ block[128:640] (fp32 PSUM
             accumulate); chunks 0-7 into accA, 8-15 into accB so accA's
             result DMA overlaps the tail of compute (host sums partials)

The PE is warmed with dense dummy matmuls from ~t=0.3us that connect
seamlessly into the real matmul stream: the HAM clock gate needs one
fully-busy 3413ns window before it opens to 2.4GHz, and any idle gap
re-arms the throttle.

Sharding: gaussians split 2048/core across 8 cores; every core accumulates
the full 128x128 image; host sums the 16 partials (2 per core), divides
num/den and reshapes to the reference's [4,3,64,64] tile layout.
"""
import numpy as np

import concourse.bacc as bacc
import concourse.tile as tile
from concourse import mybir
from concourse.bass_utils import run_bass_kernel_spmd

# Problem constants (hardcoded per harness contract)
N_GAUSS = 16384
H = 128
W = 128
FX = 128.0
FY = 128.0
CX = 64.0
CY = 64.0
EPS = 1e-8
N_CORES = 8
G_PER_CORE = N_GAUSS // N_CORES      # 2048
CHUNK = 128                          # gaussians per matmul chunk
N_CHUNKS = G_PER_CORE // CHUNK       # 16
ARG_W = 256                          # per-chunk arg width: 128 x | 128 y
OUT_W = 512                          # (c,y) free width of one accumulator

F32 = mybir.dt.float32
MM_DT = mybir.dt.float16             # coef/basis dtype for the arg matmul
BLK_DT = mybir.dt.float16            # block dtype: fp16 rounding of B is
# shared by num and den so it cancels in the ratio; colors carry an
# independent 2^-11 rounding which averages out over gaussians.
KARG = 11                            # arg-matmul contraction rows
N_WARM = 8                           # dense PE warmup matmuls (HAM gate)
BLK = 640                            # per-chunk fused block width
AW = 256                             # arg cols: x | y
PSLOT = 256                          # PSUM cols per chunk arg (1KB,
# bank-aligned so matmul outputs never cross a 2KB PSUM bank boundary)
COEF_W = G_PER_CORE + AW             # packed [basis | coef] columns


def build_program():
    """One SPMD Bass program; every core runs it on its gaussian slice."""
    nc = bacc.Bacc("TRN2", target_bir_lowering=False, debug=False,
                   num_devices=N_CORES)
    # [11, 2048+256] fp16: stationary coefficient rows (one chunk per 128
    # cols) packed with the 256 fixed basis cols so ONE DMA delivers both
    coefrhs = nc.dram_tensor("coefrhs", [KARG, COEF_W], MM_DT,
                             kind="ExternalInput")
    # [128, 64] fp32 per-gaussian color scalars
    opc = nc.dram_tensor("opc", [128, N_CHUNKS * 4], F32,
                         kind="ExternalInput")
    # two partial accumulators: [x, (den|r|g|b)*128+y] each
    out = nc.dram_tensor("out", [128, 2 * OUT_W], F32, kind="ExternalOutput")

    with tile.TileContext(nc) as tc:
        with tc.tile_pool(name="ins", bufs=1) as ins_pool, \
             tc.tile_pool(name="expp", bufs=1) as exp_pool, \
             tc.tile_pool(name="args", bufs=3, space="PSUM") as arg_pool, \
             tc.tile_pool(name="acc", bufs=1, space="PSUM") as acc_pool, \
             tc.tile_pool(name="outp", bufs=1) as out_pool:

            cr_t = ins_pool.tile([KARG, COEF_W], MM_DT)
            opc_t = ins_pool.tile([128, N_CHUNKS * 4], F32)

            # Warmup source memset on the otherwise-idle Vector engine so
            # the PE can start immediately; the critical coefrhs DMA goes
            # alone on the sync ring.  Only colrep quarter A (chunks 0-3)
            # shares the fabric with it — quarters B/C/D are gated behind a
            # tiny copy that depends on coefrhs, so their 1.2MB of traffic
            # cannot delay the matmul-critical 50KB.
            # coefrhs layout is [basis(384) | coefs(2048)]: the first DMA
            # slice carries the basis plus chunks 0-3 so the arg matmuls
            # start ~0.8us earlier; the rest follows on the same ring.
            # opc rides the scalar ring (queued before its table load).
            wsrc = ins_pool.tile([128, ARG_W], mybir.dt.bfloat16)
            nc.vector.memset(wsrc, 0.0)
            SLICE1 = AW + 4 * CHUNK
            nc.sync.dma_start(out=cr_t[:, :SLICE1], in_=coefrhs[:, :SLICE1])
            nc.sync.dma_start(out=cr_t[:, SLICE1:], in_=coefrhs[:, SLICE1:])
            nc.scalar.dma_start(out=opc_t, in_=opc[:, :])

            # fused per-chunk block [expx(128) | B(128) | colors(384)]
            t3 = exp_pool.tile([128, N_CHUNKS, BLK], BLK_DT)
            accA = acc_pool.tile([128, OUT_W], F32)
            accB = acc_pool.tile([128, OUT_W], F32)

            # Dense PE warmup into accA (overwritten by main0's start=True):
            # HAM opens the clock gate to 8/8 (2.4GHz) only after a
            # fully-busy free-running 3413ns window; these run while the
            # input DMAs are in flight and hand off to the real stream.
            for _ in range(N_WARM):
                nc.tensor.matmul(accA[:, :ARG_W], wsrc[:, :CHUNK],
                                 wsrc[:, :], start=True, stop=True)

            rhs0 = 0                               # basis cols offset
            coef0 = AW                             # coef cols offset

            def issue_args(g0c, width):
                args = arg_pool.tile([128, width, PSLOT], F32, tag="args")
                for k in range(width):
                    chunk = g0c + k
                    nc.tensor.matmul(
                        args[:, k, 0:AW],
                        cr_t[:, coef0 + chunk * CHUNK:
                             coef0 + (chunk + 1) * CHUNK],
                        cr_t[:, rhs0:rhs0 + AW],
                        start=True, stop=True,
                    )
                nc.scalar.activation(
                    out=t3[:, g0c:g0c + width, 0:AW],
                    in_=args[:, :, 0:AW],
                    func=mybir.ActivationFunctionType.Exp,
                )
                for k in range(width):
                    chunk = g0c + k
                    # y half of the exp is B = op*expy (ln(op) in the arg);
                    # color blocks multiply the SAME rounded B so num/den
                    # rounding cancels.  Columns: [den|r|g|b]
                    for c in range(3):
                        nc.vector.tensor_scalar_mul(
                            out=t3[:, chunk, 256 + c * 128:
                                   256 + (c + 1) * 128],
                            in0=t3[:, chunk, 128:256],
                            scalar1=opc_t[:, chunk * 4 + c:
                                          chunk * 4 + c + 1],
                        )

            def issue_main(chunk):
                acc = accA if chunk < 8 else accB
                nc.tensor.matmul(
                    acc[:, :],
                    t3[:, chunk, 0:128],
                    t3[:, chunk, 128:BLK],
                    start=(chunk % 8 == 0), stop=(chunk % 8 == 7),
                )

            # PE program order: args run 1-2 groups ahead of mains so the
            # engine queue always has work and never re-arms the throttle
            issue_args(0, 1)
            issue_args(1, 1)
            issue_args(2, 2)
            issue_main(0)
            issue_args(4, 4)
            for c in (1, 2, 3):
                issue_main(c)
            issue_args(8, 4)
            for c in (4, 5, 6, 7):
                issue_main(c)
            issue_args(12, 4)
            for c in range(8, 16):
                issue_main(c)

            out_t = out_pool.tile([128, 2 * OUT_W], F32)
            # accA final after pair 3: its copy+DMA overlap tail compute
            nc.scalar.copy(out=out_t[:, :OUT_W], in_=accA[:, :])
            nc.sync.dma_start(out=out[:, :OUT_W], in_=out_t[:, :OUT_W])
            # accB: halves copied on scalar+vector in parallel; scalar
            # triggers its own half's DMA (no cross-engine sem hop), sync
            # (idle) triggers the other
            nc.scalar.copy(out=out_t[:, OUT_W:OUT_W + 256],
                           in_=accB[:, :256])
            nc.scalar.dma_start(out=out[:, OUT_W:OUT_W + 256],
                                in_=out_t[:, OUT_W:OUT_W + 256])
            nc.vector.tensor_copy(out_t[:, OUT_W + 256:], accB[:, 256:])
            nc.sync.dma_start(out=out[:, OUT_W + 256:],
                              in_=out_t[:, OUT_W + 256:])

    nc.compile()
    return nc


_PROGRAM = None


def _get_program():
    global _PROGRAM
    if _PROGRAM is None:
        _PROGRAM = build_program()
    return _PROGRAM


def _quat2mat(q):
    q = q / np.linalg.norm(q)
    w, x, y, z = q
    return np.array([
        [1 - 2 * (y * y + z * z), 2 * (x * y - z * w), 2 * (x * z + y * w)],
        [2 * (x * y + z * w), 1 - 2 * (x * x + z * z), 2 * (y * z - x * w)],
        [2 * (x * z - y * w), 2 * (y * z + x * w), 1 - 2 * (x * x + y * y)],
    ])


def _hilo16(x):
    """Split x (f64) into fp16-representable hi+lo with hi+lo ~= x."""
    hi = np.asarray(x, dtype=np.float16)
    lo = (np.asarray(x, dtype=np.float64) - hi.astype(np.float64)) \
        .astype(np.float16)
    return hi, lo


def kernel(positions, colors, opacities, scales, qvec, tvec, tile_hw,
           chunk_gauss, _trace=False):
    positions = np.asarray(positions, dtype=np.float32)
    colors = np.asarray(colors, dtype=np.float32)
    opacities = np.asarray(opacities, dtype=np.float32)
    scales = np.asarray(scales, dtype=np.float32)
    qvec = np.asarray(qvec, dtype=np.float32)
    tvec = np.asarray(tvec, dtype=np.float32)
    tile_hw = int(tile_hw)
    chunk_gauss = int(chunk_gauss)
    n = positions.shape[0]
    assert n == N_GAUSS, f"expected {N_GAUSS} gaussians, got {n}"

    # ---- O(N) per-gaussian prep in float64 (rounds to the same f32 values
    # the reference computes, to well within the exp's own error budget) ----
    R = _quat2mat(qvec.astype(np.float64))
    cam = positions.astype(np.float64) @ R.T + tvec.astype(np.float64)
    ax = cam[:, 0] / cam[:, 2] * FX + CX          # [N] screen x center
    ay = cam[:, 1] / cam[:, 2] * FY + CY          # [N] screen y center
    var = scales[:, 0].astype(np.float64) ** 2
    s = -0.5 / var                                # [N] negative inv 2*var

    # centered coords keep the quadratic-expansion terms small (|u|<=64)
    dx = ax - CX
    dy = ay - CY
    op64 = opacities[:, 0].astype(np.float64)

    # K=11 stationary rows per gaussian (fp16), for args [x | y]:
    #   arg_x = s*u^2 + (-2 s dx)*u + s*dx^2            (u = x - 64)
    #   arg_y = s*v^2 + (-2 s dy)*v + s*dy^2 + ln(op)   (v = y - 64)
    # The u^2 basis is split into b1 = fp16(u^2) (exact products against
    # hi/lo halves of s) and the residual b2 = u^2-b1 in {-1,0,1} handled
    # by a single-precision s row; s_lo is pre-scaled by 2^12 (and its
    # basis row by 2^-12) to stay inside fp16 normal range.
    s_hi = s.astype(np.float16)
    s_lo12 = ((s - s_hi.astype(np.float64)) * 4096.0).astype(np.float16)
    mx_hi, mx_lo = _hilo16(-2.0 * s * dx)
    my_hi, my_lo = _hilo16(-2.0 * s * dy)
    cx_hi, cx_lo = _hilo16(s * dx * dx)
    cy_hi, cy_lo = _hilo16(s * dy * dy + np.log(op64))
    coef_full = np.stack([s_hi, s_lo12, s_hi,
                          mx_hi, mx_lo, cx_hi, cx_lo,
                          my_hi, my_lo, cy_hi, cy_lo])   # [11, N] fp16

    u = np.arange(W, dtype=np.float64) - CX
    v = np.arange(H, dtype=np.float64) - CY
    b1u = (u * u).astype(np.float16).astype(np.float64)
    b2u = u * u - b1u
    b1v = (v * v).astype(np.float16).astype(np.float64)
    b2v = v * v - b1v
    zer = np.zeros(128)
    one = np.ones(128)
    rhs_rows = [
        np.concatenate([b1u, b1v]),                   # s_hi
        np.concatenate([b1u, b1v]) / 4096.0,          # s_lo12
        np.concatenate([b2u, b2v]),                   # s_hi (residual row)
        np.concatenate([u, zer]),                     # mx_hi
        np.concatenate([u, zer]),                     # mx_lo
        np.concatenate([one, zer]),                   # cx_hi
        np.concatenate([one, zer]),                   # cx_lo
        np.concatenate([zer, v]),                     # my_hi
        np.concatenate([zer, v]),                     # my_lo
        np.concatenate([zer, one]),                   # cy_hi
        np.concatenate([zer, one]),                   # cy_lo
    ]
    rhsxy = np.stack(rhs_rows).astype(np.float16)     # [11, 256]

    col16 = colors.astype(np.float16)                 # [N, 3]
    opc_full = np.concatenate(
        [colors, np.ones((n, 1), np.float32)], axis=1).astype(np.float32)

    # ---- shard gaussians across the 8 cores ----
    in_maps = []
    for core in range(N_CORES):
        g0 = core * G_PER_CORE
        g1 = g0 + G_PER_CORE
        coefrhs = np.concatenate(
            [rhsxy, coef_full[:, g0:g1]], axis=1)     # [11, 256+2048]
        opc_c = opc_full[g0:g1].reshape(N_CHUNKS, CHUNK, 4)
        opc_c = np.ascontiguousarray(
            opc_c.transpose(1, 0, 2).reshape(CHUNK, N_CHUNKS * 4))
        in_maps.append({
            "coefrhs": np.ascontiguousarray(coefrhs),
            "opc": opc_c,
        })

    nc = _get_program()
    res = run_bass_kernel_spmd(nc, in_maps, list(range(N_CORES)),
                               trace=_trace)

    # ---- host reduction: sum per-core partials, divide, reshape ----
    acc = np.zeros((128, 4, 128), dtype=np.float64)   # [x, (den|r|g|b), y]
    for core in range(N_CORES):
        o = res.results[core]["out"]
        acc += o[:, :OUT_W].reshape(128, 4, 128)
        acc += o[:, OUT_W:].reshape(128, 4, 128)

    num = acc[:, 1:4, :]                          # [x, c, y]
    n_chunks_ref = n // chunk_gauss
    den = acc[:, 0, :] + n_chunks_ref * EPS       # [x, y]
    img = num / den[:, None, :]                   # [x, c, y]
    img = img.transpose(2, 0, 1).reshape(H * W, 3)  # [p=(y,x), c]

    step = tile_hw * tile_hw
    t = (H * W) // step
    out = img.reshape(t, step, 3).transpose(0, 2, 1).reshape(
        t, 3, tile_hw, tile_hw)
    result = out.astype(np.float32)
    if _trace:
        return result, res
    return result
